# revision 47
# baseline (speedup 1.0000x reference)
"""Trainium2 Bass kernel for a 3-layer GCN encoder (B=32, N=1000, D=256).

Math: the reference's normalized adjacency for a fully-connected graph
(self_loop=False -> adj = ones) is A_norm = ones(N,N)/N, so the
"aggregation" einsum is a mean over nodes broadcast back to every node.
Since mean o linear = linear o mean and the mean is idempotent across
layers (h is constant over nodes after layer 0), the whole network
collapses to, per batch b:

    m_b  = mean_n node_feature[b, n, :]          # (D,)
    h1_b = relu(m_b @ W0 + b0)
    h2_b = relu(h1_b @ W1 + b1)
    h3_b = h2_b @ W2 + b2
    out[b, n, :] = node_feature[b, n, :] + h3_b  # broadcast residual

Sharding: data-parallel over batch, 4 batches per core on 8 cores.

Per-core dataflow:
- Tiles use the batch-contiguous layout "(p t) d -> p t d" so every
  partition line is one contiguous 8 KB DRAM run -> 125 large DMA
  descriptors per 1 MB transfer instead of ~1000 scattered 1 KB ones.
  (Both the column-sum and the broadcast residual are row-order
  independent, so compute is unchanged by the row permutation.)
- The output is stored as bf16 (the DVE residual add casts fp32->bf16
  on write), halving HBM write traffic; the host widens back to fp32.
  Quantization error ~1e-3 relative, well under the 2e-2 gate.
- Per-batch column sums run on the PE (data as stationary, ones vector
  moving, PSUM accumulation), the 256x256 chain runs in transposed
  orientation (weights as stationary, h as a 1-column moving operand),
  bias+relu is a single DVE tensor_scalar op, the h3 broadcast across
  partitions is a rank-1 PE matmul, and the residual add reads the
  broadcast straight from PSUM with a stride-0 AP.
"""

import os

import numpy as np

import concourse.bacc as bacc
import concourse.bass as bass
import concourse.mybir as mybir
import concourse.tile as tile
from concourse.bass_utils import run_bass_kernel_spmd

F32 = mybir.dt.float32
BF16 = mybir.dt.bfloat16

B, N, D, L = 32, 1000, 256, 3
NCORES = 8
NB = B // NCORES  # batches per core
P = 125           # partition rows per node-slice
T = N // P        # node-slices per batch
HALF = 128        # half of D (partition dim for transposed chain)


def _cfg(name, default):
    return os.environ.get(name, default)


# --- A/B knobs (read at build time) ---
_V3_DEFAULT = os.environ.get("V3", "0")
LOAD_CHUNKS = int(_cfg("V2_LOAD_CHUNKS", "2"))
LOAD_ENGS = _cfg("V2_LOAD_ENGS", "sync" if _V3_DEFAULT == "1" else "sync,scalar").split(",")
STORE_CHUNKS = int(_cfg("V2_STORE_CHUNKS", "2"))
STORE_ENGS = _cfg(
    "V2_STORE_ENGS", "scalar" if _V3_DEFAULT == "1" else "gpsimd"
).split(",")
ADD_CHUNKS = int(_cfg("V2_ADD_CHUNKS", "2"))
ADD_ENGS = _cfg("V2_ADD_ENGS", "vector").split(",")
OUT_BF16 = int(_cfg("V2_OUT_BF16", "1"))
V3 = int(_cfg("V3", "0"))  # flat [128, 8000] layout, fused chain
NBUFS = int(_cfg("V2_NBUFS", "2" if V3 else "8"))
DMA_ONLY = int(_cfg("V2_DMA_ONLY", "0"))  # timing diagnostic: skip all compute
UNROLL = int(_cfg("V2_UNROLL", "1"))  # body copies per For_i iteration
PP = 128                   # v3 partitions (the 128-partition DMA fast path)
JJ = NB * N * D // PP      # 8000 = 31*256 + 64 -> phase(p) = 64*(p mod 4)
PB = PP // NB              # partitions per batch (32)

_NC_CACHE = {}


def _build_nc(reps=1):
    out_dt = BF16 if OUT_BF16 else F32

    nc = bacc.Bacc("TRN2", target_bir_lowering=False, debug=False)

    nf_d = nc.dram_tensor("nf", [NB, N, D], F32, kind="ExternalInput")
    w_d = nc.dram_tensor("w", [L, D, D], F32, kind="ExternalInput")
    bvec_d = nc.dram_tensor("bvec", [HALF, 2 * L], F32, kind="ExternalInput")
    out_d = nc.dram_tensor("out", [NB, N, D], out_dt, kind="ExternalOutput")

    ones_col_d = nc.inline_tensor(np.ones((P, 1), np.float32), "ones_col")
    ones_row_d = nc.inline_tensor(np.ones((1, P), np.float32), "ones_row")
    ident_d = nc.inline_tensor(np.eye(HALF, dtype=np.float32), "ident")
    if V3:
        # phase-major partition permutation: partition p = 32c + 8b + i holds
        # DRAM chunk j = 32b + 4i + c (8000 elems each). Every partition line
        # has constant phase 64c; class c = partitions [32c, 32c+32).
        # sel16[p, 4c+b] selects (class c, batch b) partitions.
        # (c, b) partial-sum rows live at 32c + b so every per-class slice
        # starts on a quadrant boundary (engine base-partition constraint)
        sel16_np = np.zeros((PP, PP), np.float32)
        selb_np = np.zeros((NB, PP), np.float32)
        for p in range(PP):
            c, b = p // 32, (p % 32) // 8
            sel16_np[p, 32 * c + b] = 1.0
            selb_np[b, p] = 1.0
        sel16_d = nc.inline_tensor(sel16_np, "sel16")
        selb_d = nc.inline_tensor(selb_np, "selb")
        # selc[32c+b, b'] = 1 iff b == b' (phase-class combine)
        selc_np = np.zeros((PP, NB), np.float32)
        for c in range(4):
            for b in range(NB):
                selc_np[32 * c + b, b] = 1.0
        selc_d = nc.inline_tensor(selc_np, "selc")
        ident4_d = nc.inline_tensor(np.eye(NB, dtype=np.float32), "ident4")

    add_op = mybir.AluOpType.add
    max_op = mybir.AluOpType.max

    with tile.TileContext(nc) as tc:
        with (
            tc.tile_pool(name="const", bufs=1) as cpool,
            tc.tile_pool(name="data", bufs=NBUFS) as dpool,
            tc.tile_pool(name="outb", bufs=NBUFS) as opool,
            tc.tile_pool(name="vec", bufs=8) as vpool,
            tc.tile_pool(name="fold", bufs=2) as fpool,
            tc.tile_pool(name="ps_sum", bufs=2, space=bass.MemorySpace.PSUM) as ps_sum,
            tc.tile_pool(
                name="ps_chain", bufs=1 if V3 else 2, space=bass.MemorySpace.PSUM
            ) as ps_chain,
            tc.tile_pool(name="ps_row", bufs=1, space=bass.MemorySpace.PSUM) as ps_row,
            tc.tile_pool(
                name="ps_bc", bufs=2 if V3 else 3, space=bass.MemorySpace.PSUM
            ) as ps_bc,
        ):
            # ---- constants ----
            cdma = nc.gpsimd if V3 else nc.sync
            w_sb = []
            for l in range(L):
                wt = cpool.tile([HALF, 2, D], F32, tag=f"w{l}", name=f"w{l}")
                cdma.dma_start(wt[:], w_d[l].rearrange("(kc k) e -> k kc e", k=HALF))
                w_sb.append(wt)
            bvec = cpool.tile([HALF, 2 * L], F32, tag="bvec", name="bvec")
            cdma.dma_start(bvec[:], bvec_d[:])
            ones_col = cpool.tile([P, 1], F32, tag="ones_col", name="ones_col")
            cdma.dma_start(ones_col[:], ones_col_d[:])
            ones_row = cpool.tile([1, P], F32, tag="ones_row", name="ones_row")
            cdma.dma_start(ones_row[:], ones_row_d[:])
            ident = cpool.tile([HALF, HALF], F32, tag="ident", name="ident")
            cdma.dma_start(ident[:], ident_d[:])
            consts = {"w_sb": w_sb, "bvec": bvec, "ident": ident}
            if V3:
                for nm, dten, shp in (
                    ("sel16", sel16_d, [PP, PP]),
                    ("selb", selb_d, [NB, PP]),
                    ("selc", selc_d, [PP, NB]),
                    ("ident4", ident4_d, [NB, NB]),
                ):
                    t = cpool.tile(shp, F32, tag=nm, name=nm)
                    cdma.dma_start(t[:], dten[:])
                    consts[nm] = t

            def flat_body():
                # DMA_ONLY=4: single flat [128, x] load + store, max descriptor size
                nf_flat = nf_d[:].flatten().rearrange("(p j) -> p j", p=128)
                out_flat = out_d[:].flatten().rearrange("(p j) -> p j", p=128)
                nf_t = dpool.tile([128, NB * N * D // 128], F32, tag="nfF", name="nfF")
                ob = opool.tile([128, NB * N * D // 128], out_dt, tag="obF", name="obF")
                nc.vector.memset(ob[:, 0:1], 0.0)
                step = nf_flat.shape[1] // LOAD_CHUNKS
                for s in range(LOAD_CHUNKS):
                    eng = LOAD_ENGS[s % len(LOAD_ENGS)]
                    getattr(nc, eng).dma_start(
                        nf_t[:, s * step:(s + 1) * step],
                        nf_flat[:, s * step:(s + 1) * step],
                    )
                step = out_flat.shape[1] // STORE_CHUNKS
                for s in range(STORE_CHUNKS):
                    eng = STORE_ENGS[s % len(STORE_ENGS)]
                    getattr(nc, eng).dma_start(
                        out_flat[:, s * step:(s + 1) * step],
                        ob[:, s * step:(s + 1) * step],
                    )

            def v3_body(consts):
                sel16, selc, selb, ident4, identh = (
                    consts["sel16"], consts["selc"], consts["selb"],
                    consts["ident4"], consts["ident"],
                )
                w_sb, bvec = consts["w_sb"], consts["bvec"]
                # DRAM chunk x = (b, i) at stride 32000, class c at stride 8000:
                # partition 32c + x holds flat [x*32000 + c*8000, +8000)
                nf_ch = nf_d[:].rearrange("bb n d -> (bb n d)").rearrange(
                    "(x c m) -> c x m", c=4, m=JJ
                )
                out_ch = out_d[:].rearrange("bb n d -> (bb n d)").rearrange(
                    "(x c m) -> c x m", c=4, m=JJ
                )
                F = dpool.tile([PP, JJ], F32, tag="F", name="F")
                OB = opool.tile([PP, JJ], out_dt, tag="OB", name="OB")
                # full-fabric loads: SBUF [128, m] paired with DRAM [4, 32, m]
                # by iteration order -> partition 32c + x <-> chunk x*32000+c*8000
                lstep = JJ // LOAD_CHUNKS
                for s in range(LOAD_CHUNKS):
                    eng = LOAD_ENGS[s % len(LOAD_ENGS)]
                    getattr(nc, eng).dma_start(
                        F[:, s * lstep:(s + 1) * lstep],
                        nf_ch[:, :, s * lstep:(s + 1) * lstep],
                    )

                # per-partition row-sum fold tree (all shifts are multiples of
                # 256, so the phase within each partition line is preserved):
                # 8000 = 31*256 + 64 -> s_all[p, m] = sum_k F[p, 256k + m]
                C = 256
                sc = fpool.tile([PP, 15 * C], F32, tag="sc", name="sc")
                nc.vector.tensor_add(sc[:], F[:, 0:15 * C], F[:, 15 * C:30 * C])
                nc.vector.tensor_add(sc[:, 0:C], sc[:, 0:C], F[:, 30 * C:31 * C])
                nc.vector.tensor_add(sc[:, 0:64], sc[:, 0:64], F[:, 31 * C:])
                nc.vector.tensor_add(sc[:, 0:7 * C], sc[:, 0:7 * C], sc[:, 8 * C:15 * C])
                nc.vector.tensor_add(sc[:, 0:3 * C], sc[:, 0:3 * C], sc[:, 4 * C:7 * C])
                nc.vector.tensor_add(sc[:, 0:C], sc[:, 0:C], sc[:, 3 * C:4 * C])
                nc.vector.tensor_add(sc[:, 0:C], sc[:, 0:C], sc[:, C:2 * C])
                nc.vector.tensor_add(sc[:, 0:C], sc[:, 0:C], sc[:, 2 * C:3 * C])
                nc.vector.tensor_add(sc[:, 0:C], sc[:, 0:C], sc[:, 7 * C:8 * C])

                # (class, batch) partial sums: one matmul -> psum rows 32c+b
                ps_cs = ps_sum.tile([PP, D], F32, tag="ps_cs", name="ps_cs")
                nc.tensor.matmul(
                    ps_cs[:], sel16[:], sc[:, 0:C], start=True, stop=True
                )
                # de-rotate class-c rows by 64c, then combine classes per batch
                sA = vpool.tile([PP, D], F32, tag="sA", name="sA")
                nc.vector.memset(sA[:], 0.0)
                for c in range(4):
                    rows = slice(32 * c, 32 * c + 4)
                    r = 64 * c
                    if r == 0:
                        nc.vector.tensor_copy(sA[rows, :], ps_cs[rows, :])
                    else:
                        nc.vector.tensor_copy(sA[rows, r:], ps_cs[rows, 0:D - r])
                        nc.vector.tensor_copy(sA[rows, 0:r], ps_cs[rows, D - r:])
                ps_s4 = ps_row.tile([NB, D], F32, tag="ps_s4", name="ps_s4")
                nc.tensor.matmul(ps_s4[:], selc[:], sA[:], start=True, stop=True)
                s_sb = vpool.tile([NB, D], F32, tag="s_sb", name="s_sb")
                nc.vector.tensor_copy(s_sb[:], ps_s4[:])

                # transpose sums to columns [HALF, 2, NB] (1/N is folded into W0)
                ps_h = ps_chain.tile([HALF, 2, NB], F32, tag="ps_h", name="ps_h")
                for mh in range(2):
                    nc.tensor.matmul(
                        ps_h[:, mh, :],
                        s_sb[:, mh * HALF:(mh + 1) * HALF],
                        ident4[:],
                        start=True,
                        stop=True,
                    )
                cur = vpool.tile([HALF, 2, NB], F32, tag="hc", name="h0")
                nc.vector.tensor_copy(cur[:], ps_h[:])

                # fused 3-layer chain for all batches
                for l in range(L):
                    nxt = vpool.tile([HALF, 2, NB], F32, tag="hc", name=f"h{l + 1}")
                    for mh in range(2):
                        pc = ps_chain.tile(
                            [HALF, NB], F32, tag="ps_c", name=f"ps_c{l}_{mh}"
                        )
                        for kc in range(2):
                            nc.tensor.matmul(
                                pc[:],
                                w_sb[l][:, kc, mh * HALF:(mh + 1) * HALF],
                                cur[:, kc, :],
                                start=(kc == 0),
                                stop=(kc == 1),
                            )
                        bias_ap = bvec[:, 2 * l + mh:2 * l + mh + 1]
                        if l < L - 1:
                            nc.vector.tensor_scalar(
                                nxt[:, mh, :], pc[:], bias_ap, 0.0,
                                mybir.AluOpType.add, mybir.AluOpType.max,
                            )
                        else:
                            nc.vector.tensor_scalar_add(nxt[:, mh, :], pc[:], bias_ap)
                    cur = nxt

                # transpose h3 back to rows [NB, 256]
                ps_r = ps_row.tile([NB, 2, HALF], F32, tag="ps_r", name="ps_r")
                for mh in range(2):
                    nc.tensor.matmul(
                        ps_r[:, mh, :], cur[:, mh, :], identh[:], start=True, stop=True
                    )
                # doubled rows [NB, 512] so any rotation is a linear slice
                h3d = vpool.tile([NB, 2, D], F32, tag="h3d", name="h3d")
                nc.vector.tensor_copy(
                    h3d[:].rearrange("b r (m h) -> b r m h", m=2),
                    ps_r[:].unsqueeze(1).broadcast_to([NB, 2, 2, HALF]),
                )
                # broadcast doubled rows to all partitions (one matmul), then
                # slice out each class's 64c-rotated window (4 small copies)
                pbu = ps_bc.tile([PP, 2 * D], F32, tag="ps_b", name="pbu")
                nc.tensor.matmul(pbu[:], selb[:], h3d[:], start=True, stop=True)
                bc = vpool.tile([PP, D], F32, tag="bc", name="bc")
                for c in range(4):
                    rows = slice(32 * c, 32 * (c + 1))
                    nc.vector.tensor_copy(
                        bc[rows, :], pbu[rows, 64 * c:64 * c + D]
                    )

                # residual add (fp32+fp32 -> bf16); boundaries at multiples of
                # 256 so the bc broadcast AP stays phase-aligned
                bounds = [0, 16 * C, 31 * C, JJ]
                for lo, hi in zip(bounds[:-1], bounds[1:]):
                    nk = max((hi - lo) // C, 1)
                    w = (hi - lo) // nk
                    bc_ap = bc[:, 0:w].unsqueeze(1).broadcast_to([PP, nk, w])
                    nc.vector.tensor_add(
                        OB[:, lo:hi].rearrange("p (t m) -> p t m", m=w),
                        F[:, lo:hi].rearrange("p (t m) -> p t m", m=w),
                        bc_ap,
                    )
                # full-fabric stores (same order-pairing as the loads)
                sstep = JJ // STORE_CHUNKS
                for s in range(STORE_CHUNKS):
                    eng = STORE_ENGS[s % len(STORE_ENGS)]
                    getattr(nc, eng).dma_start(
                        out_ch[:, :, s * sstep:(s + 1) * sstep],
                        OB[:, s * sstep:(s + 1) * sstep],
                    )

            def flat100_body():
                # DMA_ONLY=7: [100, 10240] load+store (phase-free layout probe)
                PP, JJ = 100, NB * N * D // 100
                nf_flat = (
                    nf_d[:].rearrange("b n d -> (b n) d")
                    .rearrange("(p t) d -> p (t d)", p=PP)
                )
                out_flat = (
                    out_d[:].rearrange("b n d -> (b n) d")
                    .rearrange("(p t) d -> p (t d)", p=PP)
                )
                nf_t = dpool.tile([PP, JJ], F32, tag="nfH", name="nfH")
                ob = opool.tile([PP, JJ], out_dt, tag="obH", name="obH")
                nc.vector.memset(ob[:, 0:1], 0.0)
                nc.sync.dma_start(nf_t[:], nf_flat)
                nc.scalar.dma_start(out_flat, ob[:])

            def flat125_body():
                # DMA_ONLY=5: single 125-partition load+store, 32KB contiguous/line
                # DMA_ONLY=6: same but 4x8KB chunks per line (batch-major)
                if DMA_ONLY == 5:
                    nf_flat = (
                        nf_d[:].rearrange("b n d -> (b n) d")
                        .rearrange("(p t) d -> p t d", p=P)
                    )
                    out_flat = (
                        out_d[:].rearrange("b n d -> (b n) d")
                        .rearrange("(p t) d -> p t d", p=P)
                    )
                else:
                    nf_flat = nf_d[:].rearrange("b (p t) d -> p b t d", p=P)
                    out_flat = out_d[:].rearrange("b (p t) d -> p b t d", p=P)
                nf_t = dpool.tile([P, NB * T, D], F32, tag="nfG", name="nfG")
                ob = opool.tile([P, NB * T, D], out_dt, tag="obG", name="obG")
                nc.vector.memset(ob[:, 0, 0:1], 0.0)
                if DMA_ONLY == 6:
                    nc.sync.dma_start(
                        nf_t[:].rearrange("p (b t) d -> p b t d", b=NB), nf_flat
                    )
                    nc.scalar.dma_start(
                        out_flat, ob[:].rearrange("p (b t) d -> p b t d", b=NB)
                    )
                else:
                    nc.sync.dma_start(nf_t[:], nf_flat)
                    nc.scalar.dma_start(out_flat, ob[:])

            def batch_body():
                if DMA_ONLY == 4:
                    flat_body()
                    return
                if DMA_ONLY in (5, 6):
                    flat125_body()
                    return
                for b in range(NB):
                    nf_t = dpool.tile([P, T, D], F32, tag="nf", name=f"nf{b}")
                    src = nf_d[b].rearrange("(p t) d -> p t d", t=T)
                    step = T // LOAD_CHUNKS
                    for s in range(LOAD_CHUNKS):
                        eng = LOAD_ENGS[s % len(LOAD_ENGS)]
                        getattr(nc, eng).dma_start(
                            nf_t[:, s * step:(s + 1) * step, :],
                            src[:, s * step:(s + 1) * step, :],
                        )

                    if DMA_ONLY:
                        # 1: loads+stores, 2: loads only, 3: stores only
                        if DMA_ONLY != 2:
                            ob = opool.tile([P, T, D], out_dt, tag="ob", name=f"ob{b}")
                            nc.vector.memset(ob[:, 0, 0:1], 0.0)
                            dst = out_d[b].rearrange("(p t) d -> p t d", t=T)
                            sstep = T // STORE_CHUNKS
                            for s in range(STORE_CHUNKS):
                                eng = STORE_ENGS[s % len(STORE_ENGS)]
                                getattr(nc, eng).dma_start(
                                    dst[:, s * sstep:(s + 1) * sstep, :],
                                    ob[:, s * sstep:(s + 1) * sstep, :],
                                )
                        continue

                    # per-batch column sums (transposed orientation):
                    # sumT[mh][d, 0] = sum_n nf[b, n, mh*128 + d]
                    h = []
                    for mh in range(2):
                        ps = ps_sum.tile([HALF, 1], F32, tag="ps_s", name=f"ps_s{b}_{mh}")
                        for t in range(T):
                            nc.tensor.matmul(
                                ps[:],
                                nf_t[:, t, mh * HALF:(mh + 1) * HALF],
                                ones_col[:],
                                start=(t == 0),
                                stop=(t == T - 1),
                            )
                        s = vpool.tile([HALF, 1], F32, tag="hT", name=f"sum{b}_{mh}")
                        nc.vector.tensor_scalar_mul(s[:], ps[:], 1.0 / N)
                        h.append(s)

                    # 3-layer chain, transposed orientation, bias+relu on DVE
                    for l in range(L):
                        hn = []
                        for mh in range(2):
                            pc = ps_chain.tile(
                                [HALF, 1], F32, tag="ps_c", name=f"ps_c{b}_{l}_{mh}"
                            )
                            for kc in range(2):
                                nc.tensor.matmul(
                                    pc[:],
                                    w_sb[l][:, kc, mh * HALF:(mh + 1) * HALF],
                                    h[kc][:],
                                    start=(kc == 0),
                                    stop=(kc == 1),
                                )
                            ht = vpool.tile([HALF, 1], F32, tag="hT", name=f"h{b}_{l}_{mh}")
                            bias_ap = bvec[:, 2 * l + mh:2 * l + mh + 1]
                            if l < L - 1:
                                nc.vector.tensor_scalar(
                                    ht[:], pc[:], bias_ap, 0.0, add_op, max_op
                                )
                            else:
                                nc.vector.tensor_scalar_add(ht[:], pc[:], bias_ap)
                            hn.append(ht)
                        h = hn

                    # transpose h3 back to a row, broadcast across partitions
                    pr = ps_row.tile([1, D], F32, tag="ps_r", name=f"ps_r{b}")
                    for kc in range(2):
                        nc.tensor.transpose(
                            pr[0:1, kc * HALF:(kc + 1) * HALF], h[kc][:], ident[:]
                        )
                    h3row = vpool.tile([1, D], F32, tag="h3row", name=f"h3row{b}")
                    nc.vector.tensor_copy(h3row[:], pr[:])
                    pb = ps_bc.tile([P, D], F32, tag="ps_b", name=f"ps_b{b}")
                    nc.tensor.matmul(pb[:], ones_row[:], h3row[:], start=True, stop=True)

                    # residual add (fp32 + fp32 -> out_dt) reading the broadcast
                    # straight from PSUM with a stride-0 AP over the t axis;
                    # store each chunk as soon as its adds complete
                    ob = opool.tile([P, T, D], out_dt, tag="ob", name=f"ob{b}")
                    dst = out_d[b].rearrange("(p t) d -> p t d", t=T)
                    astep = T // ADD_CHUNKS
                    sstep = T // STORE_CHUNKS
                    # gpsimd can't read PSUM: give it an SBUF copy of the bc row
                    bc_sb = None
                    if any(e != "vector" for e in ADD_ENGS):
                        bc_sb = vpool.tile([P, D], F32, tag="bc_sb", name=f"bc_sb{b}")
                        nc.vector.tensor_copy(bc_sb[:], pb[:])
                    adds_done = 0
                    for s in range(STORE_CHUNKS):
                        hi = (s + 1) * sstep
                        while adds_done < hi:
                            a0 = adds_done
                            eng = ADD_ENGS[(adds_done // astep) % len(ADD_ENGS)]
                            src_bc = pb if eng == "vector" else bc_sb
                            pb_bc = src_bc[:].unsqueeze(1).broadcast_to([P, astep, D])
                            getattr(nc, eng).tensor_add(
                                ob[:, a0:a0 + astep, :], nf_t[:, a0:a0 + astep, :], pb_bc
                            )
                            adds_done += astep
                        eng = STORE_ENGS[s % len(STORE_ENGS)]
                        getattr(nc, eng).dma_start(
                            dst[:, s * sstep:hi, :], ob[:, s * sstep:hi, :]
                        )

            body = (lambda: v3_body(consts)) if V3 else batch_body
            u = UNROLL if reps % UNROLL == 0 else 1
            loops = reps // u
            if loops == 1:
                for _ in range(u):
                    body()
            else:
                with tc.For_i(0, loops, 1):
                    for _ in range(u):
                        body()

    nc.compile()
    return nc


def _get_nc(reps=1):
    if reps not in _NC_CACHE:
        _NC_CACHE[reps] = _build_nc(reps)
    return _NC_CACHE[reps]


def _make_in_maps(node_feature, Ws, bs):
    nf = np.ascontiguousarray(np.asarray(node_feature, dtype=np.float32))
    w = np.ascontiguousarray(np.asarray(Ws, dtype=np.float32))
    if V3:
        w = w.copy()
        w[0] *= 1.0 / N  # fold the node-mean 1/N into W0
    b = np.asarray(bs, dtype=np.float32)
    # bvec[p, 2*l + half] = bs[l, half*128 + p]
    bvec = np.ascontiguousarray(
        b.reshape(L, 2, HALF).transpose(2, 0, 1).reshape(HALF, 2 * L)
    )
    in_maps = []
    for i in range(NCORES):
        in_maps.append(
            {
                "nf": np.ascontiguousarray(nf[i * NB:(i + 1) * NB]),
                "w": w,
                "bvec": bvec,
            }
        )
    return in_maps


def run_on_hw(node_feature, Ws, bs):
    # The NTFF trace hook (antenv.axon_hooks) does not exist in this
    # container; make sure an inherited BASS_TRACE can't pull it in.
    os.environ["BASS_NEVER_TRACE"] = "1"
    nc = _get_nc()
    res = run_bass_kernel_spmd(
        nc,
        _make_in_maps(node_feature, Ws, bs),
        list(range(NCORES)),
        trace=False,
    )
    out = np.concatenate(
        [np.asarray(res.results[i]["out"], dtype=np.float32) for i in range(NCORES)],
        axis=0,
    )
    return out, res


def kernel(x, node_feature, Ws, bs):
    node_feature = np.asarray(node_feature, dtype=np.float32)
    out, _ = run_on_hw(node_feature, Ws, bs)
    return out, node_feature


# ---------------------------------------------------------------------------
# Timing runner: same PJRT path as run_bass_kernel_spmd under axon, but with
# the jitted executable cached so repeated executions can be timed without
# re-tracing/re-compiling. Used by test.py only.
# ---------------------------------------------------------------------------


class _Runner:
    def __init__(self, nc=None):
        import jax
        from jax.experimental.shard_map import shard_map
        from jax.sharding import Mesh, NamedSharding, PartitionSpec

        from concourse.bass2jax import (
            _bass_exec_p,
            install_neuronx_cc_hook,
            partition_id_tensor,
        )

        install_neuronx_cc_hook()
        self.jax = jax
        if nc is None:
            nc = _get_nc(1)
        partition_name = (
            nc.partition_id_tensor.name if nc.partition_id_tensor else None
        )
        in_names, out_names, out_avals, zero_outs = [], [], [], []
        for alloc in nc.m.functions[0].allocations:
            if not isinstance(alloc, mybir.MemoryLocationSet):
                continue
            name = alloc.memorylocations[0].name
            if alloc.kind == "ExternalInput":
                if name != partition_name:
                    in_names.append(name)
            elif alloc.kind == "ExternalOutput":
                shape = tuple(alloc.tensor_shape)
                dt = mybir.dt.np(alloc.dtype)
                out_names.append(name)
                out_avals.append(jax.core.ShapedArray(shape, dt))
                zero_outs.append(np.zeros(shape, dt))
        self.in_names = in_names
        self.out_names = out_names
        self.out_avals = out_avals
        self.zero_outs = zero_outs
        n_params, n_outs = len(in_names), len(out_names)
        all_names = tuple(
            in_names + out_names + ([partition_name] if partition_name else [])
        )

        def _body(*args):
            operands = list(args)
            if partition_name is not None:
                operands.append(partition_id_tensor())
            outs = _bass_exec_p.bind(
                *operands,
                out_avals=tuple(out_avals),
                in_names=all_names,
                out_names=tuple(out_names),
                lowering_input_output_aliases=(),
                sim_require_finite=True,
                sim_require_nnan=True,
                nc=nc,
            )
            return tuple(outs)

        devices = jax.devices()[:NCORES]
        self.mesh = Mesh(np.asarray(devices), ("core",))
        self.sharding = NamedSharding(self.mesh, PartitionSpec("core"))
        in_specs = (PartitionSpec("core"),) * (n_params + n_outs)
        out_specs = (PartitionSpec("core"),) * n_outs
        self.jitted = jax.jit(
            shard_map(
                _body,
                mesh=self.mesh,
                in_specs=in_specs,
                out_specs=out_specs,
                check_rep=False,
            ),
            donate_argnums=tuple(range(n_params, n_params + n_outs)),
            keep_unused=True,
        )

    def stage_inputs(self, in_maps):
        concat = [
            np.concatenate([m[name] for m in in_maps], axis=0)
            for name in self.in_names
        ]
        return [self.jax.device_put(a, self.sharding) for a in concat]

    def stage_zeros(self):
        return [
            self.jax.device_put(
                np.zeros((NCORES * z.shape[0], *z.shape[1:]), z.dtype), self.sharding
            )
            for z in self.zero_outs
        ]

    def run(self, dev_inputs, dev_zeros):
        return self.jitted(*dev_inputs, *dev_zeros)


_RUNNER_CACHE = {}


def get_runner(reps=1):
    if reps not in _RUNNER_CACHE:
        _RUNNER_CACHE[reps] = _Runner(_get_nc(reps))
    return _RUNNER_CACHE[reps]


# revision 50
# speedup vs baseline: 1.2935x; 1.2935x over previous
"""Trainium2 Bass kernel for a 3-layer GCN encoder (B=32, N=1000, D=256).

Math: the reference's normalized adjacency for a fully-connected graph
(self_loop=False -> adj = ones) is A_norm = ones(N,N)/N, so the
"aggregation" einsum is a mean over nodes broadcast back to every node.
Since mean o linear = linear o mean and the mean is idempotent across
layers (h is constant over nodes after layer 0), the whole network
collapses to, per batch b:

    m_b  = mean_n node_feature[b, n, :]          # (D,)
    h1_b = relu(m_b @ W0 + b0)
    h2_b = relu(h1_b @ W1 + b1)
    h3_b = h2_b @ W2 + b2
    out[b, n, :] = node_feature[b, n, :] + h3_b  # broadcast residual

Sharding: data-parallel over batch, 4 batches per core on 8 cores.

Per-core dataflow:
- Tiles use the batch-contiguous layout "(p t) d -> p t d" so every
  partition line is one contiguous 8 KB DRAM run -> 125 large DMA
  descriptors per 1 MB transfer instead of ~1000 scattered 1 KB ones.
  (Both the column-sum and the broadcast residual are row-order
  independent, so compute is unchanged by the row permutation.)
- The output is stored as bf16 (the DVE residual add casts fp32->bf16
  on write), halving HBM write traffic; the host widens back to fp32.
  Quantization error ~1e-3 relative, well under the 2e-2 gate.
- Per-batch column sums run on the PE (data as stationary, ones vector
  moving, PSUM accumulation), the 256x256 chain runs in transposed
  orientation (weights as stationary, h as a 1-column moving operand),
  bias+relu is a single DVE tensor_scalar op, the h3 broadcast across
  partitions is a rank-1 PE matmul, and the residual add reads the
  broadcast straight from PSUM with a stride-0 AP.
"""

import os

import numpy as np

import concourse.bacc as bacc
import concourse.bass as bass
import concourse.mybir as mybir
import concourse.tile as tile
from concourse.bass_utils import run_bass_kernel_spmd

F32 = mybir.dt.float32
BF16 = mybir.dt.bfloat16

B, N, D, L = 32, 1000, 256, 3
NCORES = 8
NB = B // NCORES  # batches per core
P = 125           # partition rows per node-slice
T = N // P        # node-slices per batch
HALF = 128        # half of D (partition dim for transposed chain)


def _cfg(name, default):
    return os.environ.get(name, default)


# --- A/B knobs (read at build time) ---
_V3_DEFAULT = os.environ.get("V3", "0")
LOAD_CHUNKS = int(_cfg("V2_LOAD_CHUNKS", "2"))
LOAD_ENGS = _cfg(
    "V2_LOAD_ENGS", "sync,scalar" if _V3_DEFAULT == "0" else "sync"
).split(",")
STORE_CHUNKS = int(_cfg("V2_STORE_CHUNKS", "2"))
STORE_ENGS = _cfg(
    "V2_STORE_ENGS", "gpsimd" if _V3_DEFAULT == "0" else "scalar"
).split(",")
ADD_CHUNKS = int(_cfg("V2_ADD_CHUNKS", "2"))
ADD_ENGS = _cfg("V2_ADD_ENGS", "vector").split(",")
OUT_BF16 = int(_cfg("V2_OUT_BF16", "1"))
V3 = int(_cfg("V3", "0"))  # flat [128, 8000] layout, fused chain
NBUFS = int(_cfg("V2_NBUFS", "2" if V3 else "8"))
DMA_ONLY = int(_cfg("V2_DMA_ONLY", "0"))  # timing diagnostic: skip all compute
UNROLL = int(_cfg("V2_UNROLL", "1"))  # body copies per For_i iteration
PP = 128                   # v3 partitions (the 128-partition DMA fast path)
JJ = NB * N * D // PP      # 8000 = 31*256 + 64 -> phase(p) = 64*(p mod 4)
PB = PP // NB              # partitions per batch (32)

_NC_CACHE = {}


def _build_nc(reps=1):
    out_dt = BF16 if OUT_BF16 else F32

    nc = bacc.Bacc("TRN2", target_bir_lowering=False, debug=False)

    nf_d = nc.dram_tensor("nf", [NB, N, D], F32, kind="ExternalInput")
    w_d = nc.dram_tensor("w", [L, D, D], F32, kind="ExternalInput")
    bvec_d = nc.dram_tensor("bvec", [HALF, 2 * L], F32, kind="ExternalInput")
    out_d = nc.dram_tensor("out", [NB, N, D], out_dt, kind="ExternalOutput")

    ones_col_d = nc.inline_tensor(np.ones((P, 1), np.float32), "ones_col")
    ones_row_d = nc.inline_tensor(np.ones((1, P), np.float32), "ones_row")
    ident_d = nc.inline_tensor(np.eye(HALF, dtype=np.float32), "ident")
    if V3:
        # phase-major partition permutation: partition p = 32c + 8b + i holds
        # DRAM chunk j = 32b + 4i + c (8000 elems each). Every partition line
        # has constant phase 64c; class c = partitions [32c, 32c+32).
        # sel16[p, 4c+b] selects (class c, batch b) partitions.
        # (c, b) partial-sum rows live at 32c + b so every per-class slice
        # starts on a quadrant boundary (engine base-partition constraint)
        sel16_np = np.zeros((PP, PP), np.float32)
        selb_np = np.zeros((NB, PP), np.float32)
        for p in range(PP):
            c, b = p // 32, (p % 32) // 8
            sel16_np[p, 32 * c + b] = 1.0
            selb_np[b, p] = 1.0
        sel16_d = nc.inline_tensor(sel16_np, "sel16")
        selb_d = nc.inline_tensor(selb_np, "selb")
        # selc[32c+b, b'] = 1 iff b == b' (phase-class combine)
        selc_np = np.zeros((PP, NB), np.float32)
        for c in range(4):
            for b in range(NB):
                selc_np[32 * c + b, b] = 1.0
        selc_d = nc.inline_tensor(selc_np, "selc")
        ident4_d = nc.inline_tensor(np.eye(NB, dtype=np.float32), "ident4")
        # v4 (V3=2): unequal 256-aligned split, batch(p) = p//24 (p<96) else (p-96)//8
        def _v4_batch(p):
            return p // 24 if p < 96 else (p - 96) // 8
        selB2_np = np.zeros((PP, NB), np.float32)
        selbT_np = np.zeros((NB, PP), np.float32)
        for p in range(PP):
            selB2_np[p, _v4_batch(p)] = 1.0
            selbT_np[_v4_batch(p), p] = 1.0
        selB2_d = nc.inline_tensor(selB2_np, "selB2")
        selbT_d = nc.inline_tensor(selbT_np, "selbT")

    add_op = mybir.AluOpType.add
    max_op = mybir.AluOpType.max

    with tile.TileContext(nc) as tc:
        with (
            tc.tile_pool(name="const", bufs=1) as cpool,
            tc.tile_pool(name="data", bufs=NBUFS) as dpool,
            tc.tile_pool(name="outb", bufs=NBUFS) as opool,
            tc.tile_pool(name="vec", bufs=8) as vpool,
            tc.tile_pool(name="fold", bufs=2) as fpool,
            tc.tile_pool(name="ps_sum", bufs=2, space=bass.MemorySpace.PSUM) as ps_sum,
            tc.tile_pool(
                name="ps_chain", bufs=1 if V3 else 2, space=bass.MemorySpace.PSUM
            ) as ps_chain,
            tc.tile_pool(name="ps_row", bufs=1, space=bass.MemorySpace.PSUM) as ps_row,
            tc.tile_pool(
                name="ps_bc", bufs=2 if V3 else 3, space=bass.MemorySpace.PSUM
            ) as ps_bc,
        ):
            # ---- constants ----
            cdma = nc.gpsimd if V3 else nc.sync
            w_sb = []
            for l in range(L):
                wt = cpool.tile([HALF, 2, D], F32, tag=f"w{l}", name=f"w{l}")
                cdma.dma_start(wt[:], w_d[l].rearrange("(kc k) e -> k kc e", k=HALF))
                w_sb.append(wt)
            bvec = cpool.tile([HALF, 2 * L], F32, tag="bvec", name="bvec")
            cdma.dma_start(bvec[:], bvec_d[:])
            ones_col = cpool.tile([P, 1], F32, tag="ones_col", name="ones_col")
            cdma.dma_start(ones_col[:], ones_col_d[:])
            ones_row = cpool.tile([1, P], F32, tag="ones_row", name="ones_row")
            cdma.dma_start(ones_row[:], ones_row_d[:])
            ident = cpool.tile([HALF, HALF], F32, tag="ident", name="ident")
            cdma.dma_start(ident[:], ident_d[:])
            consts = {"w_sb": w_sb, "bvec": bvec, "ident": ident}
            if V3:
                for nm, dten, shp in (
                    ("sel16", sel16_d, [PP, PP]),
                    ("selb", selb_d, [NB, PP]),
                    ("selc", selc_d, [PP, NB]),
                    ("ident4", ident4_d, [NB, NB]),
                    ("selB2", selB2_d, [PP, NB]),
                    ("selbT", selbT_d, [NB, PP]),
                ):
                    t = cpool.tile(shp, F32, tag=nm, name=nm)
                    cdma.dma_start(t[:], dten[:])
                    consts[nm] = t

            def flat_body():
                # DMA_ONLY=4: single flat [128, x] load + store, max descriptor size
                nf_flat = nf_d[:].flatten().rearrange("(p j) -> p j", p=128)
                out_flat = out_d[:].flatten().rearrange("(p j) -> p j", p=128)
                nf_t = dpool.tile([128, NB * N * D // 128], F32, tag="nfF", name="nfF")
                ob = opool.tile([128, NB * N * D // 128], out_dt, tag="obF", name="obF")
                nc.vector.memset(ob[:, 0:1], 0.0)
                step = nf_flat.shape[1] // LOAD_CHUNKS
                for s in range(LOAD_CHUNKS):
                    eng = LOAD_ENGS[s % len(LOAD_ENGS)]
                    getattr(nc, eng).dma_start(
                        nf_t[:, s * step:(s + 1) * step],
                        nf_flat[:, s * step:(s + 1) * step],
                    )
                step = out_flat.shape[1] // STORE_CHUNKS
                for s in range(STORE_CHUNKS):
                    eng = STORE_ENGS[s % len(STORE_ENGS)]
                    getattr(nc, eng).dma_start(
                        out_flat[:, s * step:(s + 1) * step],
                        ob[:, s * step:(s + 1) * step],
                    )

            def v3_body(consts):
                sel16, selc, selb, ident4, identh = (
                    consts["sel16"], consts["selc"], consts["selb"],
                    consts["ident4"], consts["ident"],
                )
                w_sb, bvec = consts["w_sb"], consts["bvec"]
                # DRAM chunk x = (b, i) at stride 32000, class c at stride 8000:
                # partition 32c + x holds flat [x*32000 + c*8000, +8000)
                nf_ch = nf_d[:].rearrange("bb n d -> (bb n d)").rearrange(
                    "(x c m) -> c x m", c=4, m=JJ
                )
                out_ch = out_d[:].rearrange("bb n d -> (bb n d)").rearrange(
                    "(x c m) -> c x m", c=4, m=JJ
                )
                F = dpool.tile([PP, JJ], F32, tag="F", name="F")
                OB = opool.tile([PP, JJ], out_dt, tag="OB", name="OB")
                # full-fabric loads: SBUF [128, m] paired with DRAM [4, 32, m]
                # by iteration order -> partition 32c + x <-> chunk x*32000+c*8000
                lstep = JJ // LOAD_CHUNKS
                for s in range(LOAD_CHUNKS):
                    eng = LOAD_ENGS[s % len(LOAD_ENGS)]
                    getattr(nc, eng).dma_start(
                        F[:, s * lstep:(s + 1) * lstep],
                        nf_ch[:, :, s * lstep:(s + 1) * lstep],
                    )

                # per-partition row-sum fold tree (all shifts are multiples of
                # 256, so the phase within each partition line is preserved):
                # 8000 = 31*256 + 64 -> s_all[p, m] = sum_k F[p, 256k + m]
                C = 256
                sc = fpool.tile([PP, 15 * C], F32, tag="sc", name="sc")
                nc.vector.tensor_add(sc[:], F[:, 0:15 * C], F[:, 15 * C:30 * C])
                nc.vector.tensor_add(sc[:, 0:C], sc[:, 0:C], F[:, 30 * C:31 * C])
                nc.vector.tensor_add(sc[:, 0:64], sc[:, 0:64], F[:, 31 * C:])
                nc.vector.tensor_add(sc[:, 0:7 * C], sc[:, 0:7 * C], sc[:, 8 * C:15 * C])
                nc.vector.tensor_add(sc[:, 0:3 * C], sc[:, 0:3 * C], sc[:, 4 * C:7 * C])
                nc.vector.tensor_add(sc[:, 0:C], sc[:, 0:C], sc[:, 3 * C:4 * C])
                nc.vector.tensor_add(sc[:, 0:C], sc[:, 0:C], sc[:, C:2 * C])
                nc.vector.tensor_add(sc[:, 0:C], sc[:, 0:C], sc[:, 2 * C:3 * C])
                nc.vector.tensor_add(sc[:, 0:C], sc[:, 0:C], sc[:, 7 * C:8 * C])

                # (class, batch) partial sums: one matmul -> psum rows 32c+b
                ps_cs = ps_sum.tile([PP, D], F32, tag="ps_cs", name="ps_cs")
                nc.tensor.matmul(
                    ps_cs[:], sel16[:], sc[:, 0:C], start=True, stop=True
                )
                # de-rotate class-c rows by 64c, then combine classes per batch
                sA = vpool.tile([PP, D], F32, tag="sA", name="sA")
                nc.vector.memset(sA[:], 0.0)
                for c in range(4):
                    rows = slice(32 * c, 32 * c + 4)
                    r = 64 * c
                    if r == 0:
                        nc.vector.tensor_copy(sA[rows, :], ps_cs[rows, :])
                    else:
                        nc.vector.tensor_copy(sA[rows, r:], ps_cs[rows, 0:D - r])
                        nc.vector.tensor_copy(sA[rows, 0:r], ps_cs[rows, D - r:])
                ps_s4 = ps_row.tile([NB, D], F32, tag="ps_s4", name="ps_s4")
                nc.tensor.matmul(ps_s4[:], selc[:], sA[:], start=True, stop=True)
                s_sb = vpool.tile([NB, D], F32, tag="s_sb", name="s_sb")
                nc.vector.tensor_copy(s_sb[:], ps_s4[:])

                # transpose sums to columns [HALF, 2, NB] (1/N is folded into W0)
                ps_h = ps_chain.tile([HALF, 2, NB], F32, tag="ps_h", name="ps_h")
                for mh in range(2):
                    nc.tensor.matmul(
                        ps_h[:, mh, :],
                        s_sb[:, mh * HALF:(mh + 1) * HALF],
                        ident4[:],
                        start=True,
                        stop=True,
                    )
                cur = vpool.tile([HALF, 2, NB], F32, tag="hc", name="h0")
                nc.vector.tensor_copy(cur[:], ps_h[:])

                # fused 3-layer chain for all batches
                for l in range(L):
                    nxt = vpool.tile([HALF, 2, NB], F32, tag="hc", name=f"h{l + 1}")
                    for mh in range(2):
                        pc = ps_chain.tile(
                            [HALF, NB], F32, tag="ps_c", name=f"ps_c{l}_{mh}"
                        )
                        for kc in range(2):
                            nc.tensor.matmul(
                                pc[:],
                                w_sb[l][:, kc, mh * HALF:(mh + 1) * HALF],
                                cur[:, kc, :],
                                start=(kc == 0),
                                stop=(kc == 1),
                            )
                        bias_ap = bvec[:, 2 * l + mh:2 * l + mh + 1]
                        if l < L - 1:
                            nc.vector.tensor_scalar(
                                nxt[:, mh, :], pc[:], bias_ap, 0.0,
                                mybir.AluOpType.add, mybir.AluOpType.max,
                            )
                        else:
                            nc.vector.tensor_scalar_add(nxt[:, mh, :], pc[:], bias_ap)
                    cur = nxt

                # transpose h3 back to rows [NB, 256]
                ps_r = ps_row.tile([NB, 2, HALF], F32, tag="ps_r", name="ps_r")
                for mh in range(2):
                    nc.tensor.matmul(
                        ps_r[:, mh, :], cur[:, mh, :], identh[:], start=True, stop=True
                    )
                # doubled rows [NB, 512] so any rotation is a linear slice
                h3d = vpool.tile([NB, 2, D], F32, tag="h3d", name="h3d")
                nc.vector.tensor_copy(
                    h3d[:].rearrange("b r (m h) -> b r m h", m=2),
                    ps_r[:].unsqueeze(1).broadcast_to([NB, 2, 2, HALF]),
                )
                # broadcast doubled rows to all partitions (one matmul), then
                # slice out each class's 64c-rotated window (4 small copies)
                pbu = ps_bc.tile([PP, 2 * D], F32, tag="ps_b", name="pbu")
                nc.tensor.matmul(pbu[:], selb[:], h3d[:], start=True, stop=True)
                bc = vpool.tile([PP, D], F32, tag="bc", name="bc")
                for c in range(4):
                    rows = slice(32 * c, 32 * (c + 1))
                    nc.vector.tensor_copy(
                        bc[rows, :], pbu[rows, 64 * c:64 * c + D]
                    )

                # residual add (fp32+fp32 -> bf16); boundaries at multiples of
                # 256 so the bc broadcast AP stays phase-aligned
                bounds = [0, 16 * C, 31 * C, JJ]
                for lo, hi in zip(bounds[:-1], bounds[1:]):
                    nk = max((hi - lo) // C, 1)
                    w = (hi - lo) // nk
                    bc_ap = bc[:, 0:w].unsqueeze(1).broadcast_to([PP, nk, w])
                    nc.vector.tensor_add(
                        OB[:, lo:hi].rearrange("p (t m) -> p t m", m=w),
                        F[:, lo:hi].rearrange("p (t m) -> p t m", m=w),
                        bc_ap,
                    )
                # full-fabric stores (same order-pairing as the loads)
                sstep = JJ // STORE_CHUNKS
                for s in range(STORE_CHUNKS):
                    eng = STORE_ENGS[s % len(STORE_ENGS)]
                    getattr(nc, eng).dma_start(
                        out_ch[:, :, s * sstep:(s + 1) * sstep],
                        OB[:, s * sstep:(s + 1) * sstep],
                    )

            def v4_body(consts):
                # V3=2: unequal 256-aligned flat split. Partitions 0..95 hold
                # 31 rows (7936 elems), 96..127 hold 32 rows (8192); per batch
                # 24+8 partitions. No phase rotation anywhere.
                selB2, selbT, ident4, identh = (
                    consts["selB2"], consts["selbT"], consts["ident4"],
                    consts["ident"],
                )
                w_sb, bvec = consts["w_sb"], consts["bvec"]
                C = 256
                JA, JB = 31 * C, 32 * C
                flat = nf_d[:].rearrange("bb n d -> (bb n d)")
                oflat = out_d[:].rearrange("bb n d -> (bb n d)")
                # A: 96 lines of 7936 from offset b*256000 + q*7936
                nfA = flat[0:1024000].rearrange(
                    "(b r) -> b r", b=NB
                )[:, 0:96 // NB * JA].rearrange("b (q m) -> b q m", m=JA)
                outA = oflat[0:1024000].rearrange(
                    "(b r) -> b r", b=NB
                )[:, 0:96 // NB * JA].rearrange("b (q m) -> b q m", m=JA)
                # B: 32 lines of 8192 from offset b*256000 + 24*7936 + r*8192
                nfB = flat[0:1024000].rearrange(
                    "(b r) -> b r", b=NB
                )[:, 96 // NB * JA:].rearrange("b (q m) -> b q m", m=JB)
                outB = oflat[0:1024000].rearrange(
                    "(b r) -> b r", b=NB
                )[:, 96 // NB * JA:].rearrange("b (q m) -> b q m", m=JB)

                F = dpool.tile([PP, JB], F32, tag="F", name="F")
                OB = opool.tile([PP, JB], out_dt, tag="OB", name="OB")
                engA = LOAD_ENGS[0]
                engB = LOAD_ENGS[1 % len(LOAD_ENGS)]
                getattr(nc, engA).dma_start(F[0:96, 0:JA], nfA)
                getattr(nc, engB).dma_start(F[96:PP, 0:JB], nfB)

                # fold trees (all shifts multiples of 256)
                sc = fpool.tile([PP, 16 * C], F32, tag="sc", name="sc")
                va = nc.vector
                # A group: 31 chunks -> 1
                va.tensor_add(sc[0:96, 0:15 * C], F[0:96, 0:15 * C], F[0:96, 15 * C:30 * C])
                va.tensor_add(sc[0:96, 0:7 * C], sc[0:96, 0:7 * C], sc[0:96, 8 * C:15 * C])
                va.tensor_add(sc[0:96, 0:3 * C], sc[0:96, 0:3 * C], sc[0:96, 4 * C:7 * C])
                va.tensor_add(sc[0:96, 0:C], sc[0:96, 0:C], sc[0:96, C:2 * C])
                va.tensor_add(sc[0:96, 0:C], sc[0:96, 0:C], sc[0:96, 2 * C:3 * C])
                va.tensor_add(sc[0:96, 0:C], sc[0:96, 0:C], sc[0:96, 3 * C:4 * C])
                va.tensor_add(sc[0:96, 0:C], sc[0:96, 0:C], sc[0:96, 7 * C:8 * C])
                va.tensor_add(sc[0:96, 0:C], sc[0:96, 0:C], F[0:96, 30 * C:31 * C])
                # B group: 32 chunks -> 1
                va.tensor_add(sc[96:PP, 0:16 * C], F[96:PP, 0:16 * C], F[96:PP, 16 * C:32 * C])
                va.tensor_add(sc[96:PP, 0:8 * C], sc[96:PP, 0:8 * C], sc[96:PP, 8 * C:16 * C])
                va.tensor_add(sc[96:PP, 0:4 * C], sc[96:PP, 0:4 * C], sc[96:PP, 4 * C:8 * C])
                va.tensor_add(sc[96:PP, 0:2 * C], sc[96:PP, 0:2 * C], sc[96:PP, 2 * C:4 * C])
                va.tensor_add(sc[96:PP, 0:C], sc[96:PP, 0:C], sc[96:PP, C:2 * C])

                # per-batch column sums: one matmul
                ps_s4 = ps_row.tile([NB, D], F32, tag="ps_s4", name="ps_s4")
                nc.tensor.matmul(ps_s4[:], selB2[:], sc[:, 0:C], start=True, stop=True)
                s_sb = vpool.tile([NB, D], F32, tag="s_sb", name="s_sb")
                nc.vector.tensor_copy(s_sb[:], ps_s4[:])

                # transpose sums to columns [HALF, 2, NB] (1/N folded into W0)
                ps_h = ps_chain.tile([HALF, 2, NB], F32, tag="ps_h", name="ps_h")
                for mh in range(2):
                    nc.tensor.matmul(
                        ps_h[:, mh, :],
                        s_sb[:, mh * HALF:(mh + 1) * HALF],
                        ident4[:],
                        start=True,
                        stop=True,
                    )
                cur = vpool.tile([HALF, 2, NB], F32, tag="hc", name="h0")
                nc.vector.tensor_copy(cur[:], ps_h[:])

                for l in range(L):
                    nxt = vpool.tile([HALF, 2, NB], F32, tag="hc", name=f"h{l + 1}")
                    for mh in range(2):
                        pc = ps_chain.tile(
                            [HALF, NB], F32, tag="ps_c", name=f"ps_c{l}_{mh}"
                        )
                        for kc in range(2):
                            nc.tensor.matmul(
                                pc[:],
                                w_sb[l][:, kc, mh * HALF:(mh + 1) * HALF],
                                cur[:, kc, :],
                                start=(kc == 0),
                                stop=(kc == 1),
                            )
                        bias_ap = bvec[:, 2 * l + mh:2 * l + mh + 1]
                        if l < L - 1:
                            nc.vector.tensor_scalar(
                                nxt[:, mh, :], pc[:], bias_ap, 0.0,
                                mybir.AluOpType.add, mybir.AluOpType.max,
                            )
                        else:
                            nc.vector.tensor_scalar_add(nxt[:, mh, :], pc[:], bias_ap)
                    cur = nxt

                # h3 back to rows [NB, 256], then broadcast to all partitions
                ps_r = ps_row.tile([NB, 2, HALF], F32, tag="ps_r", name="ps_r")
                for mh in range(2):
                    nc.tensor.matmul(
                        ps_r[:, mh, :], cur[:, mh, :], identh[:], start=True, stop=True
                    )
                h3_sb = vpool.tile([NB, 2, HALF], F32, tag="h3", name="h3_sb")
                nc.vector.tensor_copy(h3_sb[:], ps_r[:])
                bc = ps_bc.tile([PP, D], F32, tag="ps_b", name="bc")
                nc.tensor.matmul(
                    bc[:], selbT[:], h3_sb[:].rearrange("b m h -> b (m h)"),
                    start=True, stop=True,
                )

                # residual adds reading bc straight from PSUM (stride-0 t axis)
                for lo, hi, base, npart in (
                    (0, 16, 0, 96), (16, 31, 0, 96),
                    (0, 16, 96, 32), (16, 32, 96, 32),
                ):
                    nk = hi - lo
                    bc_ap = bc[base:base + npart, :].unsqueeze(1).broadcast_to(
                        [npart, nk, D]
                    )
                    sl = slice(base, base + npart)
                    nc.vector.tensor_add(
                        OB[sl, lo * C:hi * C].rearrange("p (t m) -> p t m", m=C),
                        F[sl, lo * C:hi * C].rearrange("p (t m) -> p t m", m=C),
                        bc_ap,
                    )
                engA = STORE_ENGS[0]
                engB = STORE_ENGS[1 % len(STORE_ENGS)]
                getattr(nc, engA).dma_start(outA, OB[0:96, 0:JA])
                getattr(nc, engB).dma_start(outB, OB[96:PP, 0:JB])

            def flat100_body():
                # DMA_ONLY=7: [100, 10240] load+store (phase-free layout probe)
                PP, JJ = 100, NB * N * D // 100
                nf_flat = (
                    nf_d[:].rearrange("b n d -> (b n) d")
                    .rearrange("(p t) d -> p (t d)", p=PP)
                )
                out_flat = (
                    out_d[:].rearrange("b n d -> (b n) d")
                    .rearrange("(p t) d -> p (t d)", p=PP)
                )
                nf_t = dpool.tile([PP, JJ], F32, tag="nfH", name="nfH")
                ob = opool.tile([PP, JJ], out_dt, tag="obH", name="obH")
                nc.vector.memset(ob[:, 0:1], 0.0)
                nc.sync.dma_start(nf_t[:], nf_flat)
                nc.scalar.dma_start(out_flat, ob[:])

            def flat125_body():
                # DMA_ONLY=5: single 125-partition load+store, 32KB contiguous/line
                # DMA_ONLY=6: same but 4x8KB chunks per line (batch-major)
                if DMA_ONLY == 5:
                    nf_flat = (
                        nf_d[:].rearrange("b n d -> (b n) d")
                        .rearrange("(p t) d -> p t d", p=P)
                    )
                    out_flat = (
                        out_d[:].rearrange("b n d -> (b n) d")
                        .rearrange("(p t) d -> p t d", p=P)
                    )
                else:
                    nf_flat = nf_d[:].rearrange("b (p t) d -> p b t d", p=P)
                    out_flat = out_d[:].rearrange("b (p t) d -> p b t d", p=P)
                nf_t = dpool.tile([P, NB * T, D], F32, tag="nfG", name="nfG")
                ob = opool.tile([P, NB * T, D], out_dt, tag="obG", name="obG")
                nc.vector.memset(ob[:, 0, 0:1], 0.0)
                if DMA_ONLY == 6:
                    nc.sync.dma_start(
                        nf_t[:].rearrange("p (b t) d -> p b t d", b=NB), nf_flat
                    )
                    nc.scalar.dma_start(
                        out_flat, ob[:].rearrange("p (b t) d -> p b t d", b=NB)
                    )
                else:
                    nc.sync.dma_start(nf_t[:], nf_flat)
                    nc.scalar.dma_start(out_flat, ob[:])

            def batch_body():
                if DMA_ONLY == 4:
                    flat_body()
                    return
                if DMA_ONLY in (5, 6):
                    flat125_body()
                    return
                for b in range(NB):
                    nf_t = dpool.tile([P, T, D], F32, tag="nf", name=f"nf{b}")
                    src = nf_d[b].rearrange("(p t) d -> p t d", t=T)
                    step = T // LOAD_CHUNKS
                    for s in range(LOAD_CHUNKS):
                        eng = LOAD_ENGS[s % len(LOAD_ENGS)]
                        getattr(nc, eng).dma_start(
                            nf_t[:, s * step:(s + 1) * step, :],
                            src[:, s * step:(s + 1) * step, :],
                        )

                    if DMA_ONLY:
                        # 1: loads+stores, 2: loads only, 3: stores only
                        if DMA_ONLY != 2:
                            ob = opool.tile([P, T, D], out_dt, tag="ob", name=f"ob{b}")
                            nc.vector.memset(ob[:, 0, 0:1], 0.0)
                            dst = out_d[b].rearrange("(p t) d -> p t d", t=T)
                            sstep = T // STORE_CHUNKS
                            for s in range(STORE_CHUNKS):
                                eng = STORE_ENGS[s % len(STORE_ENGS)]
                                getattr(nc, eng).dma_start(
                                    dst[:, s * sstep:(s + 1) * sstep, :],
                                    ob[:, s * sstep:(s + 1) * sstep, :],
                                )
                        continue

                    # per-batch column sums (transposed orientation):
                    # sumT[mh][d, 0] = sum_n nf[b, n, mh*128 + d]
                    h = []
                    for mh in range(2):
                        ps = ps_sum.tile([HALF, 1], F32, tag="ps_s", name=f"ps_s{b}_{mh}")
                        for t in range(T):
                            nc.tensor.matmul(
                                ps[:],
                                nf_t[:, t, mh * HALF:(mh + 1) * HALF],
                                ones_col[:],
                                start=(t == 0),
                                stop=(t == T - 1),
                            )
                        s = vpool.tile([HALF, 1], F32, tag="hT", name=f"sum{b}_{mh}")
                        nc.vector.tensor_scalar_mul(s[:], ps[:], 1.0 / N)
                        h.append(s)

                    # 3-layer chain, transposed orientation, bias+relu on DVE
                    for l in range(L):
                        hn = []
                        for mh in range(2):
                            pc = ps_chain.tile(
                                [HALF, 1], F32, tag="ps_c", name=f"ps_c{b}_{l}_{mh}"
                            )
                            for kc in range(2):
                                nc.tensor.matmul(
                                    pc[:],
                                    w_sb[l][:, kc, mh * HALF:(mh + 1) * HALF],
                                    h[kc][:],
                                    start=(kc == 0),
                                    stop=(kc == 1),
                                )
                            ht = vpool.tile([HALF, 1], F32, tag="hT", name=f"h{b}_{l}_{mh}")
                            bias_ap = bvec[:, 2 * l + mh:2 * l + mh + 1]
                            if l < L - 1:
                                nc.vector.tensor_scalar(
                                    ht[:], pc[:], bias_ap, 0.0, add_op, max_op
                                )
                            else:
                                nc.vector.tensor_scalar_add(ht[:], pc[:], bias_ap)
                            hn.append(ht)
                        h = hn

                    # transpose h3 back to a row, broadcast across partitions
                    pr = ps_row.tile([1, D], F32, tag="ps_r", name=f"ps_r{b}")
                    for kc in range(2):
                        nc.tensor.transpose(
                            pr[0:1, kc * HALF:(kc + 1) * HALF], h[kc][:], ident[:]
                        )
                    h3row = vpool.tile([1, D], F32, tag="h3row", name=f"h3row{b}")
                    nc.vector.tensor_copy(h3row[:], pr[:])
                    pb = ps_bc.tile([P, D], F32, tag="ps_b", name=f"ps_b{b}")
                    nc.tensor.matmul(pb[:], ones_row[:], h3row[:], start=True, stop=True)

                    # residual add (fp32 + fp32 -> out_dt) reading the broadcast
                    # straight from PSUM with a stride-0 AP over the t axis;
                    # store each chunk as soon as its adds complete
                    ob = opool.tile([P, T, D], out_dt, tag="ob", name=f"ob{b}")
                    dst = out_d[b].rearrange("(p t) d -> p t d", t=T)
                    astep = T // ADD_CHUNKS
                    sstep = T // STORE_CHUNKS
                    # gpsimd can't read PSUM: give it an SBUF copy of the bc row
                    bc_sb = None
                    if any(e != "vector" for e in ADD_ENGS):
                        bc_sb = vpool.tile([P, D], F32, tag="bc_sb", name=f"bc_sb{b}")
                        nc.vector.tensor_copy(bc_sb[:], pb[:])
                    adds_done = 0
                    for s in range(STORE_CHUNKS):
                        hi = (s + 1) * sstep
                        while adds_done < hi:
                            a0 = adds_done
                            eng = ADD_ENGS[(adds_done // astep) % len(ADD_ENGS)]
                            src_bc = pb if eng == "vector" else bc_sb
                            pb_bc = src_bc[:].unsqueeze(1).broadcast_to([P, astep, D])
                            getattr(nc, eng).tensor_add(
                                ob[:, a0:a0 + astep, :], nf_t[:, a0:a0 + astep, :], pb_bc
                            )
                            adds_done += astep
                        eng = STORE_ENGS[s % len(STORE_ENGS)]
                        getattr(nc, eng).dma_start(
                            dst[:, s * sstep:hi, :], ob[:, s * sstep:hi, :]
                        )

            if V3 == 2:
                body = lambda: v4_body(consts)
            elif V3:
                body = lambda: v3_body(consts)
            else:
                body = batch_body
            u = UNROLL if reps % UNROLL == 0 else 1
            loops = reps // u
            if loops == 1:
                for _ in range(u):
                    body()
            else:
                with tc.For_i(0, loops, 1):
                    for _ in range(u):
                        body()

    nc.compile()
    return nc


def _get_nc(reps=1):
    if reps not in _NC_CACHE:
        _NC_CACHE[reps] = _build_nc(reps)
    return _NC_CACHE[reps]


def _make_in_maps(node_feature, Ws, bs):
    nf = np.ascontiguousarray(np.asarray(node_feature, dtype=np.float32))
    w = np.ascontiguousarray(np.asarray(Ws, dtype=np.float32))
    if V3:
        w = w.copy()
        w[0] *= 1.0 / N  # fold the node-mean 1/N into W0
    b = np.asarray(bs, dtype=np.float32)
    # bvec[p, 2*l + half] = bs[l, half*128 + p]
    bvec = np.ascontiguousarray(
        b.reshape(L, 2, HALF).transpose(2, 0, 1).reshape(HALF, 2 * L)
    )
    in_maps = []
    for i in range(NCORES):
        in_maps.append(
            {
                "nf": np.ascontiguousarray(nf[i * NB:(i + 1) * NB]),
                "w": w,
                "bvec": bvec,
            }
        )
    return in_maps


def run_on_hw(node_feature, Ws, bs):
    # The NTFF trace hook (antenv.axon_hooks) does not exist in this
    # container; make sure an inherited BASS_TRACE can't pull it in.
    os.environ["BASS_NEVER_TRACE"] = "1"
    nc = _get_nc()
    res = run_bass_kernel_spmd(
        nc,
        _make_in_maps(node_feature, Ws, bs),
        list(range(NCORES)),
        trace=False,
    )
    out = np.concatenate(
        [np.asarray(res.results[i]["out"], dtype=np.float32) for i in range(NCORES)],
        axis=0,
    )
    return out, res


def kernel(x, node_feature, Ws, bs):
    node_feature = np.asarray(node_feature, dtype=np.float32)
    out, _ = run_on_hw(node_feature, Ws, bs)
    return out, node_feature


# ---------------------------------------------------------------------------
# Timing runner: same PJRT path as run_bass_kernel_spmd under axon, but with
# the jitted executable cached so repeated executions can be timed without
# re-tracing/re-compiling. Used by test.py only.
# ---------------------------------------------------------------------------


class _Runner:
    def __init__(self, nc=None):
        import jax
        from jax.experimental.shard_map import shard_map
        from jax.sharding import Mesh, NamedSharding, PartitionSpec

        from concourse.bass2jax import (
            _bass_exec_p,
            install_neuronx_cc_hook,
            partition_id_tensor,
        )

        install_neuronx_cc_hook()
        self.jax = jax
        if nc is None:
            nc = _get_nc(1)
        partition_name = (
            nc.partition_id_tensor.name if nc.partition_id_tensor else None
        )
        in_names, out_names, out_avals, zero_outs = [], [], [], []
        for alloc in nc.m.functions[0].allocations:
            if not isinstance(alloc, mybir.MemoryLocationSet):
                continue
            name = alloc.memorylocations[0].name
            if alloc.kind == "ExternalInput":
                if name != partition_name:
                    in_names.append(name)
            elif alloc.kind == "ExternalOutput":
                shape = tuple(alloc.tensor_shape)
                dt = mybir.dt.np(alloc.dtype)
                out_names.append(name)
                out_avals.append(jax.core.ShapedArray(shape, dt))
                zero_outs.append(np.zeros(shape, dt))
        self.in_names = in_names
        self.out_names = out_names
        self.out_avals = out_avals
        self.zero_outs = zero_outs
        n_params, n_outs = len(in_names), len(out_names)
        all_names = tuple(
            in_names + out_names + ([partition_name] if partition_name else [])
        )

        def _body(*args):
            operands = list(args)
            if partition_name is not None:
                operands.append(partition_id_tensor())
            outs = _bass_exec_p.bind(
                *operands,
                out_avals=tuple(out_avals),
                in_names=all_names,
                out_names=tuple(out_names),
                lowering_input_output_aliases=(),
                sim_require_finite=True,
                sim_require_nnan=True,
                nc=nc,
            )
            return tuple(outs)

        devices = jax.devices()[:NCORES]
        self.mesh = Mesh(np.asarray(devices), ("core",))
        self.sharding = NamedSharding(self.mesh, PartitionSpec("core"))
        in_specs = (PartitionSpec("core"),) * (n_params + n_outs)
        out_specs = (PartitionSpec("core"),) * n_outs
        self.jitted = jax.jit(
            shard_map(
                _body,
                mesh=self.mesh,
                in_specs=in_specs,
                out_specs=out_specs,
                check_rep=False,
            ),
            donate_argnums=tuple(range(n_params, n_params + n_outs)),
            keep_unused=True,
        )

    def stage_inputs(self, in_maps):
        concat = [
            np.concatenate([m[name] for m in in_maps], axis=0)
            for name in self.in_names
        ]
        return [self.jax.device_put(a, self.sharding) for a in concat]

    def stage_zeros(self):
        return [
            self.jax.device_put(
                np.zeros((NCORES * z.shape[0], *z.shape[1:]), z.dtype), self.sharding
            )
            for z in self.zero_outs
        ]

    def run(self, dev_inputs, dev_zeros):
        return self.jitted(*dev_inputs, *dev_zeros)


_RUNNER_CACHE = {}


def get_runner(reps=1):
    if reps not in _RUNNER_CACHE:
        _RUNNER_CACHE[reps] = _Runner(_get_nc(reps))
    return _RUNNER_CACHE[reps]


# revision 53
# speedup vs baseline: 2.0536x; 1.5877x over previous
"""Trainium2 Bass kernel for a 3-layer GCN encoder (B=32, N=1000, D=256).

Math: the reference's normalized adjacency for a fully-connected graph
(self_loop=False -> adj = ones) is A_norm = ones(N,N)/N, so the
"aggregation" einsum is a mean over nodes broadcast back to every node.
Since mean o linear = linear o mean and the mean is idempotent across
layers (h is constant over nodes after layer 0), the whole network
collapses to, per batch b:

    m_b  = mean_n node_feature[b, n, :]          # (D,)
    h1_b = relu(m_b @ W0 + b0)
    h2_b = relu(h1_b @ W1 + b1)
    h3_b = h2_b @ W2 + b2
    out[b, n, :] = node_feature[b, n, :] + h3_b  # broadcast residual

Sharding: data-parallel over batch, 4 batches per core on 8 cores.

Per-core dataflow:
- Tiles use the batch-contiguous layout "(p t) d -> p t d" so every
  partition line is one contiguous 8 KB DRAM run -> 125 large DMA
  descriptors per 1 MB transfer instead of ~1000 scattered 1 KB ones.
  (Both the column-sum and the broadcast residual are row-order
  independent, so compute is unchanged by the row permutation.)
- The output is stored as bf16 (the DVE residual add casts fp32->bf16
  on write), halving HBM write traffic; the host widens back to fp32.
  Quantization error ~1e-3 relative, well under the 2e-2 gate.
- Per-batch column sums run on the PE (data as stationary, ones vector
  moving, PSUM accumulation), the 256x256 chain runs in transposed
  orientation (weights as stationary, h as a 1-column moving operand),
  bias+relu is a single DVE tensor_scalar op, the h3 broadcast across
  partitions is a rank-1 PE matmul, and the residual add reads the
  broadcast straight from PSUM with a stride-0 AP.
"""

import os

import numpy as np

import concourse.bacc as bacc
import concourse.bass as bass
import concourse.mybir as mybir
import concourse.tile as tile
from concourse.bass_utils import run_bass_kernel_spmd

F32 = mybir.dt.float32
BF16 = mybir.dt.bfloat16

B, N, D, L = 32, 1000, 256, 3
NCORES = 8
NB = B // NCORES  # batches per core
P = 125           # partition rows per node-slice
T = N // P        # node-slices per batch
HALF = 128        # half of D (partition dim for transposed chain)


def _cfg(name, default):
    return os.environ.get(name, default)


# --- A/B knobs (read at build time) ---
_V3_DEFAULT = os.environ.get("V3", "0")
LOAD_CHUNKS = int(_cfg("V2_LOAD_CHUNKS", "2"))
LOAD_ENGS = _cfg(
    "V2_LOAD_ENGS", "sync,scalar" if _V3_DEFAULT == "0" else "sync"
).split(",")
STORE_CHUNKS = int(_cfg("V2_STORE_CHUNKS", "2"))
STORE_ENGS = _cfg(
    "V2_STORE_ENGS", "gpsimd" if _V3_DEFAULT == "0" else "scalar"
).split(",")
ADD_CHUNKS = int(_cfg("V2_ADD_CHUNKS", "2"))
ADD_ENGS = _cfg("V2_ADD_ENGS", "vector").split(",")
OUT_BF16 = int(_cfg("V2_OUT_BF16", "1"))
V3 = int(_cfg("V3", "0"))  # flat [128, 8000] layout, fused chain
NBUFS = int(_cfg("V2_NBUFS", "2" if V3 else "8"))
DMA_ONLY = int(_cfg("V2_DMA_ONLY", "0"))  # timing diagnostic: skip all compute
UNROLL = int(_cfg("V2_UNROLL", "1"))  # body copies per For_i iteration
PP = 128                   # v3 partitions (the 128-partition DMA fast path)
JJ = NB * N * D // PP      # 8000 = 31*256 + 64 -> phase(p) = 64*(p mod 4)
PB = PP // NB              # partitions per batch (32)

_NC_CACHE = {}


def _build_nc(reps=1):
    out_dt = BF16 if OUT_BF16 else F32

    nc = bacc.Bacc("TRN2", target_bir_lowering=False, debug=False)

    nf_d = nc.dram_tensor("nf", [NB, N, D], F32, kind="ExternalInput")
    w_d = nc.dram_tensor("w", [L, D, D], F32, kind="ExternalInput")
    bvec_d = nc.dram_tensor("bvec", [HALF, 2 * L], F32, kind="ExternalInput")
    out_d = nc.dram_tensor("out", [NB, N, D], out_dt, kind="ExternalOutput")

    ones_col_d = nc.inline_tensor(np.ones((P, 1), np.float32), "ones_col")
    ones_row_d = nc.inline_tensor(np.ones((1, P), np.float32), "ones_row")
    ident_d = nc.inline_tensor(np.eye(HALF, dtype=np.float32), "ident")
    if V3:
        # phase-major partition permutation: partition p = 32c + 8b + i holds
        # DRAM chunk j = 32b + 4i + c (8000 elems each). Every partition line
        # has constant phase 64c; class c = partitions [32c, 32c+32).
        # sel16[p, 4c+b] selects (class c, batch b) partitions.
        # (c, b) partial-sum rows live at 32c + b so every per-class slice
        # starts on a quadrant boundary (engine base-partition constraint)
        sel16_np = np.zeros((PP, PP), np.float32)
        selb_np = np.zeros((NB, PP), np.float32)
        for p in range(PP):
            c, b = p // 32, (p % 32) // 8
            sel16_np[p, 32 * c + b] = 1.0
            selb_np[b, p] = 1.0
        sel16_d = nc.inline_tensor(sel16_np, "sel16")
        selb_d = nc.inline_tensor(selb_np, "selb")
        # selc[32c+b, b'] = 1 iff b == b' (phase-class combine)
        selc_np = np.zeros((PP, NB), np.float32)
        for c in range(4):
            for b in range(NB):
                selc_np[32 * c + b, b] = 1.0
        selc_d = nc.inline_tensor(selc_np, "selc")
        ident4_d = nc.inline_tensor(np.eye(NB, dtype=np.float32), "ident4")
        # v4 (V3=2): unequal 256-aligned split, batch(p) = p//24 (p<96) else (p-96)//8
        def _v4_batch(p):
            return p // 24 if p < 96 else (p - 96) // 8
        selB2_np = np.zeros((PP, NB), np.float32)
        selbT_np = np.zeros((NB, PP), np.float32)
        for p in range(PP):
            selB2_np[p, _v4_batch(p)] = 1.0
            selbT_np[_v4_batch(p), p] = 1.0
        selB2_d = nc.inline_tensor(selB2_np, "selB2")
        selbT_d = nc.inline_tensor(selbT_np, "selbT")
        # v5 (V3=3): equal flat split, c(p) = p mod 4, b(p) = p // 32;
        # (c, b) rows live at 32c + b (quadrant bases)
        sel16v5_np = np.zeros((PP, PP), np.float32)
        selrot_np = np.zeros((PP, PP), np.float32)
        for p in range(PP):
            r = 32 * (p % 4) + p // 32
            sel16v5_np[p, r] = 1.0   # colsum: partition p -> row r
            selrot_np[r, p] = 1.0    # bc: row r -> partition p
        sel16v5_d = nc.inline_tensor(sel16v5_np, "sel16v5")
        selrot_d = nc.inline_tensor(selrot_np, "selrot")
        # stat36[b, 32+b] = 1: writes class-3 rotated rows at out rows 96..99
        # via a base-64 PE output (PE out base must be 0/32/64)
        stat36_np = np.zeros((NB, 36), np.float32)
        for b in range(NB):
            stat36_np[b, 32 + b] = 1.0
        stat36_d = nc.inline_tensor(stat36_np, "stat36")

    add_op = mybir.AluOpType.add
    max_op = mybir.AluOpType.max

    with tile.TileContext(nc) as tc:
        with (
            tc.tile_pool(name="const", bufs=1) as cpool,
            tc.tile_pool(name="data", bufs=NBUFS) as dpool,
            tc.tile_pool(name="outb", bufs=NBUFS) as opool,
            tc.tile_pool(name="vec", bufs=8) as vpool,
            tc.tile_pool(name="fold", bufs=2) as fpool,
            tc.tile_pool(name="ps_sum", bufs=2, space=bass.MemorySpace.PSUM) as ps_sum,
            tc.tile_pool(
                name="ps_chain", bufs=1 if V3 else 2, space=bass.MemorySpace.PSUM
            ) as ps_chain,
            tc.tile_pool(name="ps_row", bufs=1, space=bass.MemorySpace.PSUM) as ps_row,
            tc.tile_pool(
                name="ps_bc", bufs=(1 if V3 == 3 else 2) if V3 else 3,
                space=bass.MemorySpace.PSUM
            ) as ps_bc,
        ):
            # ---- constants ----
            cdma = nc.gpsimd if V3 else nc.sync
            w_sb = []
            for l in range(L):
                wt = cpool.tile([HALF, 2, D], F32, tag=f"w{l}", name=f"w{l}")
                cdma.dma_start(wt[:], w_d[l].rearrange("(kc k) e -> k kc e", k=HALF))
                w_sb.append(wt)
            bvec = cpool.tile([HALF, 2 * L], F32, tag="bvec", name="bvec")
            cdma.dma_start(bvec[:], bvec_d[:])
            ones_col = cpool.tile([P, 1], F32, tag="ones_col", name="ones_col")
            cdma.dma_start(ones_col[:], ones_col_d[:])
            ones_row = cpool.tile([1, P], F32, tag="ones_row", name="ones_row")
            cdma.dma_start(ones_row[:], ones_row_d[:])
            ident = cpool.tile([HALF, HALF], F32, tag="ident", name="ident")
            cdma.dma_start(ident[:], ident_d[:])
            consts = {"w_sb": w_sb, "bvec": bvec, "ident": ident}
            if V3:
                for nm, dten, shp in (
                    ("sel16", sel16_d, [PP, PP]),
                    ("selb", selb_d, [NB, PP]),
                    ("selc", selc_d, [PP, NB]),
                    ("ident4", ident4_d, [NB, NB]),
                    ("selB2", selB2_d, [PP, NB]),
                    ("selbT", selbT_d, [NB, PP]),
                    ("sel16v5", sel16v5_d, [PP, PP]),
                    ("selrot", selrot_d, [PP, PP]),
                    ("stat36", stat36_d, [NB, 36]),
                ):
                    t = cpool.tile(shp, F32, tag=nm, name=nm)
                    cdma.dma_start(t[:], dten[:])
                    consts[nm] = t

            def flat_body():
                # DMA_ONLY=4: single flat [128, x] load + store, max descriptor size
                nf_flat = nf_d[:].flatten().rearrange("(p j) -> p j", p=128)
                out_flat = out_d[:].flatten().rearrange("(p j) -> p j", p=128)
                nf_t = dpool.tile([128, NB * N * D // 128], F32, tag="nfF", name="nfF")
                ob = opool.tile([128, NB * N * D // 128], out_dt, tag="obF", name="obF")
                nc.vector.memset(ob[:, 0:1], 0.0)
                step = nf_flat.shape[1] // LOAD_CHUNKS
                for s in range(LOAD_CHUNKS):
                    eng = LOAD_ENGS[s % len(LOAD_ENGS)]
                    getattr(nc, eng).dma_start(
                        nf_t[:, s * step:(s + 1) * step],
                        nf_flat[:, s * step:(s + 1) * step],
                    )
                step = out_flat.shape[1] // STORE_CHUNKS
                for s in range(STORE_CHUNKS):
                    eng = STORE_ENGS[s % len(STORE_ENGS)]
                    getattr(nc, eng).dma_start(
                        out_flat[:, s * step:(s + 1) * step],
                        ob[:, s * step:(s + 1) * step],
                    )

            def v3_body(consts):
                sel16, selc, selb, ident4, identh = (
                    consts["sel16"], consts["selc"], consts["selb"],
                    consts["ident4"], consts["ident"],
                )
                w_sb, bvec = consts["w_sb"], consts["bvec"]
                # DRAM chunk x = (b, i) at stride 32000, class c at stride 8000:
                # partition 32c + x holds flat [x*32000 + c*8000, +8000)
                nf_ch = nf_d[:].rearrange("bb n d -> (bb n d)").rearrange(
                    "(x c m) -> c x m", c=4, m=JJ
                )
                out_ch = out_d[:].rearrange("bb n d -> (bb n d)").rearrange(
                    "(x c m) -> c x m", c=4, m=JJ
                )
                F = dpool.tile([PP, JJ], F32, tag="F", name="F")
                OB = opool.tile([PP, JJ], out_dt, tag="OB", name="OB")
                # full-fabric loads: SBUF [128, m] paired with DRAM [4, 32, m]
                # by iteration order -> partition 32c + x <-> chunk x*32000+c*8000
                lstep = JJ // LOAD_CHUNKS
                for s in range(LOAD_CHUNKS):
                    eng = LOAD_ENGS[s % len(LOAD_ENGS)]
                    getattr(nc, eng).dma_start(
                        F[:, s * lstep:(s + 1) * lstep],
                        nf_ch[:, :, s * lstep:(s + 1) * lstep],
                    )

                # per-partition row-sum fold tree (all shifts are multiples of
                # 256, so the phase within each partition line is preserved):
                # 8000 = 31*256 + 64 -> s_all[p, m] = sum_k F[p, 256k + m]
                C = 256
                sc = fpool.tile([PP, 15 * C], F32, tag="sc", name="sc")
                nc.vector.tensor_add(sc[:], F[:, 0:15 * C], F[:, 15 * C:30 * C])
                nc.vector.tensor_add(sc[:, 0:C], sc[:, 0:C], F[:, 30 * C:31 * C])
                nc.vector.tensor_add(sc[:, 0:64], sc[:, 0:64], F[:, 31 * C:])
                nc.vector.tensor_add(sc[:, 0:7 * C], sc[:, 0:7 * C], sc[:, 8 * C:15 * C])
                nc.vector.tensor_add(sc[:, 0:3 * C], sc[:, 0:3 * C], sc[:, 4 * C:7 * C])
                nc.vector.tensor_add(sc[:, 0:C], sc[:, 0:C], sc[:, 3 * C:4 * C])
                nc.vector.tensor_add(sc[:, 0:C], sc[:, 0:C], sc[:, C:2 * C])
                nc.vector.tensor_add(sc[:, 0:C], sc[:, 0:C], sc[:, 2 * C:3 * C])
                nc.vector.tensor_add(sc[:, 0:C], sc[:, 0:C], sc[:, 7 * C:8 * C])

                # (class, batch) partial sums: one matmul -> psum rows 32c+b
                ps_cs = ps_sum.tile([PP, D], F32, tag="ps_cs", name="ps_cs")
                nc.tensor.matmul(
                    ps_cs[:], sel16[:], sc[:, 0:C], start=True, stop=True
                )
                # de-rotate class-c rows by 64c, then combine classes per batch
                sA = vpool.tile([PP, D], F32, tag="sA", name="sA")
                nc.vector.memset(sA[:], 0.0)
                for c in range(4):
                    rows = slice(32 * c, 32 * c + 4)
                    r = 64 * c
                    if r == 0:
                        nc.vector.tensor_copy(sA[rows, :], ps_cs[rows, :])
                    else:
                        nc.vector.tensor_copy(sA[rows, r:], ps_cs[rows, 0:D - r])
                        nc.vector.tensor_copy(sA[rows, 0:r], ps_cs[rows, D - r:])
                ps_s4 = ps_row.tile([NB, D], F32, tag="ps_s4", name="ps_s4")
                nc.tensor.matmul(ps_s4[:], selc[:], sA[:], start=True, stop=True)
                s_sb = vpool.tile([NB, D], F32, tag="s_sb", name="s_sb")
                nc.vector.tensor_copy(s_sb[:], ps_s4[:])

                # transpose sums to columns [HALF, 2, NB] (1/N is folded into W0)
                ps_h = ps_chain.tile([HALF, 2, NB], F32, tag="ps_h", name="ps_h")
                for mh in range(2):
                    nc.tensor.matmul(
                        ps_h[:, mh, :],
                        s_sb[:, mh * HALF:(mh + 1) * HALF],
                        ident4[:],
                        start=True,
                        stop=True,
                    )
                cur = vpool.tile([HALF, 2, NB], F32, tag="hc", name="h0")
                nc.vector.tensor_copy(cur[:], ps_h[:])

                # fused 3-layer chain for all batches
                for l in range(L):
                    nxt = vpool.tile([HALF, 2, NB], F32, tag="hc", name=f"h{l + 1}")
                    for mh in range(2):
                        pc = ps_chain.tile(
                            [HALF, NB], F32, tag="ps_c", name=f"ps_c{l}_{mh}"
                        )
                        for kc in range(2):
                            nc.tensor.matmul(
                                pc[:],
                                w_sb[l][:, kc, mh * HALF:(mh + 1) * HALF],
                                cur[:, kc, :],
                                start=(kc == 0),
                                stop=(kc == 1),
                            )
                        bias_ap = bvec[:, 2 * l + mh:2 * l + mh + 1]
                        if l < L - 1:
                            nc.vector.tensor_scalar(
                                nxt[:, mh, :], pc[:], bias_ap, 0.0,
                                mybir.AluOpType.add, mybir.AluOpType.max,
                            )
                        else:
                            nc.vector.tensor_scalar_add(nxt[:, mh, :], pc[:], bias_ap)
                    cur = nxt

                # transpose h3 back to rows [NB, 256]
                ps_r = ps_row.tile([NB, 2, HALF], F32, tag="ps_r", name="ps_r")
                for mh in range(2):
                    nc.tensor.matmul(
                        ps_r[:, mh, :], cur[:, mh, :], identh[:], start=True, stop=True
                    )
                # doubled rows [NB, 512] so any rotation is a linear slice
                h3d = vpool.tile([NB, 2, D], F32, tag="h3d", name="h3d")
                nc.vector.tensor_copy(
                    h3d[:].rearrange("b r (m h) -> b r m h", m=2),
                    ps_r[:].unsqueeze(1).broadcast_to([NB, 2, 2, HALF]),
                )
                # broadcast doubled rows to all partitions (one matmul), then
                # slice out each class's 64c-rotated window (4 small copies)
                pbu = ps_bc.tile([PP, 2 * D], F32, tag="ps_b", name="pbu")
                nc.tensor.matmul(pbu[:], selb[:], h3d[:], start=True, stop=True)
                bc = vpool.tile([PP, D], F32, tag="bc", name="bc")
                for c in range(4):
                    rows = slice(32 * c, 32 * (c + 1))
                    nc.vector.tensor_copy(
                        bc[rows, :], pbu[rows, 64 * c:64 * c + D]
                    )

                # residual add (fp32+fp32 -> bf16); boundaries at multiples of
                # 256 so the bc broadcast AP stays phase-aligned
                bounds = [0, 16 * C, 31 * C, JJ]
                for lo, hi in zip(bounds[:-1], bounds[1:]):
                    nk = max((hi - lo) // C, 1)
                    w = (hi - lo) // nk
                    bc_ap = bc[:, 0:w].unsqueeze(1).broadcast_to([PP, nk, w])
                    nc.vector.tensor_add(
                        OB[:, lo:hi].rearrange("p (t m) -> p t m", m=w),
                        F[:, lo:hi].rearrange("p (t m) -> p t m", m=w),
                        bc_ap,
                    )
                # full-fabric stores (same order-pairing as the loads)
                sstep = JJ // STORE_CHUNKS
                for s in range(STORE_CHUNKS):
                    eng = STORE_ENGS[s % len(STORE_ENGS)]
                    getattr(nc, eng).dma_start(
                        out_ch[:, :, s * sstep:(s + 1) * sstep],
                        OB[:, s * sstep:(s + 1) * sstep],
                    )

            def v5_body(consts):
                # V3=3: equal flat split [128, 8000], STRICTLY 2D same-shape
                # DMAs (the only fast path), phases handled by selectors.
                sel16, selc, selrot, ident4, identh = (
                    consts["sel16v5"], consts["selc"], consts["selrot"],
                    consts["ident4"], consts["ident"],
                )
                w_sb, bvec = consts["w_sb"], consts["bvec"]
                C = 256
                nf_flat = (
                    nf_d[:].rearrange("bb n d -> (bb n d)")
                    .rearrange("(p j) -> p j", p=PP)
                )
                out_flat = (
                    out_d[:].rearrange("bb n d -> (bb n d)")
                    .rearrange("(p j) -> p j", p=PP)
                )
                F = dpool.tile([PP, JJ], F32, tag="F", name="F")
                OB = opool.tile([PP, JJ], out_dt, tag="OB", name="OB")
                lstep = JJ // LOAD_CHUNKS
                for s in range(LOAD_CHUNKS):
                    eng = LOAD_ENGS[s % len(LOAD_ENGS)]
                    getattr(nc, eng).dma_start(
                        F[:, s * lstep:(s + 1) * lstep],
                        nf_flat[:, s * lstep:(s + 1) * lstep],
                    )

                # per-partition fold tree (shifts all multiples of 256):
                # 8000 = 31*256 + 64
                sc = fpool.tile([PP, 15 * C], F32, tag="sc", name="sc")
                va = nc.vector
                va.tensor_add(sc[:], F[:, 0:15 * C], F[:, 15 * C:30 * C])
                va.tensor_add(sc[:, 0:C], sc[:, 0:C], F[:, 30 * C:31 * C])
                va.tensor_add(sc[:, 0:64], sc[:, 0:64], F[:, 31 * C:])
                va.tensor_add(sc[:, 0:7 * C], sc[:, 0:7 * C], sc[:, 8 * C:15 * C])
                va.tensor_add(sc[:, 0:3 * C], sc[:, 0:3 * C], sc[:, 4 * C:7 * C])
                va.tensor_add(sc[:, 0:C], sc[:, 0:C], sc[:, C:2 * C])
                va.tensor_add(sc[:, 0:C], sc[:, 0:C], sc[:, 2 * C:3 * C])
                va.tensor_add(sc[:, 0:C], sc[:, 0:C], sc[:, 3 * C:4 * C])
                va.tensor_add(sc[:, 0:C], sc[:, 0:C], sc[:, 7 * C:8 * C])

                # (class, batch) partial sums at rows 32c + b
                ps_cs = ps_sum.tile([PP, D], F32, tag="ps_cs", name="ps_cs")
                nc.tensor.matmul(
                    ps_cs[:], sel16[:], sc[:, 0:C], start=True, stop=True
                )
                # de-rotate class-c rows by 64c, combine classes per batch
                sA = vpool.tile([PP, D], F32, tag="sA", name="sA")
                nc.vector.memset(sA[:], 0.0)
                for c in range(4):
                    rows = slice(32 * c, 32 * c + 4)
                    r = 64 * c
                    if r == 0:
                        nc.vector.tensor_copy(sA[rows, :], ps_cs[rows, :])
                    else:
                        nc.vector.tensor_copy(sA[rows, r:], ps_cs[rows, 0:D - r])
                        nc.vector.tensor_copy(sA[rows, 0:r], ps_cs[rows, D - r:])
                ps_s4 = ps_row.tile([NB, D], F32, tag="ps_s4", name="ps_s4")
                nc.tensor.matmul(ps_s4[:], selc[:], sA[:], start=True, stop=True)
                s_sb = vpool.tile([NB, D], F32, tag="s_sb", name="s_sb")
                nc.vector.tensor_copy(s_sb[:], ps_s4[:])

                # transpose sums to columns [HALF, 2, NB] (1/N folded into W0)
                ps_h = ps_chain.tile([HALF, 2, NB], F32, tag="ps_h", name="ps_h")
                for mh in range(2):
                    nc.tensor.matmul(
                        ps_h[:, mh, :],
                        s_sb[:, mh * HALF:(mh + 1) * HALF],
                        ident4[:],
                        start=True,
                        stop=True,
                    )
                cur = vpool.tile([HALF, 2, NB], F32, tag="hc", name="h0")
                nc.vector.tensor_copy(cur[:], ps_h[:])

                for l in range(L):
                    nxt = vpool.tile([HALF, 2, NB], F32, tag="hc", name=f"h{l + 1}")
                    for mh in range(2):
                        pc = ps_chain.tile(
                            [HALF, NB], F32, tag="ps_c", name=f"ps_c{l}_{mh}"
                        )
                        for kc in range(2):
                            nc.tensor.matmul(
                                pc[:],
                                w_sb[l][:, kc, mh * HALF:(mh + 1) * HALF],
                                cur[:, kc, :],
                                start=(kc == 0),
                                stop=(kc == 1),
                            )
                        bias_ap = bvec[:, 2 * l + mh:2 * l + mh + 1]
                        if l < L - 1:
                            nc.vector.tensor_scalar(
                                nxt[:, mh, :], pc[:], bias_ap, 0.0,
                                mybir.AluOpType.add, mybir.AluOpType.max,
                            )
                        else:
                            nc.vector.tensor_scalar_add(nxt[:, mh, :], pc[:], bias_ap)
                    cur = nxt

                # h3 -> rows [NB, 256] -> doubled [NB, 512]
                ps_r = ps_row.tile([NB, 2, HALF], F32, tag="ps_r", name="ps_r")
                for mh in range(2):
                    nc.tensor.matmul(
                        ps_r[:, mh, :], cur[:, mh, :], identh[:], start=True, stop=True
                    )
                h3d = vpool.tile([NB, 2, D], F32, tag="h3d", name="h3d")
                nc.vector.tensor_copy(
                    h3d[:].rearrange("b r (m h) -> b r m h", m=2),
                    ps_r[:].unsqueeze(1).broadcast_to([NB, 2, 2, HALF]),
                )
                # rotated rows (c, b) at 32c + b: 4 small PE matmuls
                ps_rot = ps_bc.tile([PP, D], F32, tag="ps_rot", name="ps_rot")
                h3dr = h3d[:].rearrange("b r m -> b (r m)")
                # c=3 first: base-64 out spanning rows 64..99 (rows 96..99 live)
                nc.tensor.matmul(
                    ps_rot[64:100, :], consts["stat36"][:],
                    h3dr[:, 192:192 + D], start=True, stop=True,
                )
                for c in range(3):
                    nc.tensor.matmul(
                        ps_rot[32 * c:32 * c + 4, :],
                        ident4[:],
                        h3dr[:, 64 * c:64 * c + D],
                        start=True,
                        stop=True,
                    )
                h3rot = vpool.tile([PP, D], F32, tag="h3rot", name="h3rot")
                nc.vector.memset(h3rot[:], 0.0)
                for c in range(4):
                    rows = slice(32 * c, 32 * c + 4)
                    nc.vector.tensor_copy(h3rot[rows, :], ps_rot[rows, :])
                # bc[p] = rot_{64(p mod 4)}(h3row[p // 32]): selector matmul
                bc = ps_bc.tile([PP, D], F32, tag="ps_b", name="bc")
                nc.tensor.matmul(bc[:], selrot[:], h3rot[:], start=True, stop=True)

                # residual adds reading bc straight from PSUM (stride-0 t axis)
                bounds = [0, 16 * C, 31 * C, JJ]
                for lo, hi in zip(bounds[:-1], bounds[1:]):
                    nk = max((hi - lo) // C, 1)
                    w = (hi - lo) // nk
                    bc_ap = bc[:, 0:w].unsqueeze(1).broadcast_to([PP, nk, w])
                    nc.vector.tensor_add(
                        OB[:, lo:hi].rearrange("p (t m) -> p t m", m=w),
                        F[:, lo:hi].rearrange("p (t m) -> p t m", m=w),
                        bc_ap,
                    )
                sstep = JJ // STORE_CHUNKS
                for s in range(STORE_CHUNKS):
                    eng = STORE_ENGS[s % len(STORE_ENGS)]
                    getattr(nc, eng).dma_start(
                        out_flat[:, s * sstep:(s + 1) * sstep],
                        OB[:, s * sstep:(s + 1) * sstep],
                    )

            def v4_body(consts):
                # V3=2: unequal 256-aligned flat split. Partitions 0..95 hold
                # 31 rows (7936 elems), 96..127 hold 32 rows (8192); per batch
                # 24+8 partitions. No phase rotation anywhere.
                selB2, selbT, ident4, identh = (
                    consts["selB2"], consts["selbT"], consts["ident4"],
                    consts["ident"],
                )
                w_sb, bvec = consts["w_sb"], consts["bvec"]
                C = 256
                JA, JB = 31 * C, 32 * C
                flat = nf_d[:].rearrange("bb n d -> (bb n d)")
                oflat = out_d[:].rearrange("bb n d -> (bb n d)")
                # A: 96 lines of 7936 from offset b*256000 + q*7936
                nfA = flat[0:1024000].rearrange(
                    "(b r) -> b r", b=NB
                )[:, 0:96 // NB * JA].rearrange("b (q m) -> b q m", m=JA)
                outA = oflat[0:1024000].rearrange(
                    "(b r) -> b r", b=NB
                )[:, 0:96 // NB * JA].rearrange("b (q m) -> b q m", m=JA)
                # B: 32 lines of 8192 from offset b*256000 + 24*7936 + r*8192
                nfB = flat[0:1024000].rearrange(
                    "(b r) -> b r", b=NB
                )[:, 96 // NB * JA:].rearrange("b (q m) -> b q m", m=JB)
                outB = oflat[0:1024000].rearrange(
                    "(b r) -> b r", b=NB
                )[:, 96 // NB * JA:].rearrange("b (q m) -> b q m", m=JB)

                F = dpool.tile([PP, JB], F32, tag="F", name="F")
                OB = opool.tile([PP, JB], out_dt, tag="OB", name="OB")
                engA = LOAD_ENGS[0]
                engB = LOAD_ENGS[1 % len(LOAD_ENGS)]
                getattr(nc, engA).dma_start(F[0:96, 0:JA], nfA)
                getattr(nc, engB).dma_start(F[96:PP, 0:JB], nfB)

                # fold trees (all shifts multiples of 256)
                sc = fpool.tile([PP, 16 * C], F32, tag="sc", name="sc")
                va = nc.vector
                # A group: 31 chunks -> 1
                va.tensor_add(sc[0:96, 0:15 * C], F[0:96, 0:15 * C], F[0:96, 15 * C:30 * C])
                va.tensor_add(sc[0:96, 0:7 * C], sc[0:96, 0:7 * C], sc[0:96, 8 * C:15 * C])
                va.tensor_add(sc[0:96, 0:3 * C], sc[0:96, 0:3 * C], sc[0:96, 4 * C:7 * C])
                va.tensor_add(sc[0:96, 0:C], sc[0:96, 0:C], sc[0:96, C:2 * C])
                va.tensor_add(sc[0:96, 0:C], sc[0:96, 0:C], sc[0:96, 2 * C:3 * C])
                va.tensor_add(sc[0:96, 0:C], sc[0:96, 0:C], sc[0:96, 3 * C:4 * C])
                va.tensor_add(sc[0:96, 0:C], sc[0:96, 0:C], sc[0:96, 7 * C:8 * C])
                va.tensor_add(sc[0:96, 0:C], sc[0:96, 0:C], F[0:96, 30 * C:31 * C])
                # B group: 32 chunks -> 1
                va.tensor_add(sc[96:PP, 0:16 * C], F[96:PP, 0:16 * C], F[96:PP, 16 * C:32 * C])
                va.tensor_add(sc[96:PP, 0:8 * C], sc[96:PP, 0:8 * C], sc[96:PP, 8 * C:16 * C])
                va.tensor_add(sc[96:PP, 0:4 * C], sc[96:PP, 0:4 * C], sc[96:PP, 4 * C:8 * C])
                va.tensor_add(sc[96:PP, 0:2 * C], sc[96:PP, 0:2 * C], sc[96:PP, 2 * C:4 * C])
                va.tensor_add(sc[96:PP, 0:C], sc[96:PP, 0:C], sc[96:PP, C:2 * C])

                # per-batch column sums: one matmul
                ps_s4 = ps_row.tile([NB, D], F32, tag="ps_s4", name="ps_s4")
                nc.tensor.matmul(ps_s4[:], selB2[:], sc[:, 0:C], start=True, stop=True)
                s_sb = vpool.tile([NB, D], F32, tag="s_sb", name="s_sb")
                nc.vector.tensor_copy(s_sb[:], ps_s4[:])

                # transpose sums to columns [HALF, 2, NB] (1/N folded into W0)
                ps_h = ps_chain.tile([HALF, 2, NB], F32, tag="ps_h", name="ps_h")
                for mh in range(2):
                    nc.tensor.matmul(
                        ps_h[:, mh, :],
                        s_sb[:, mh * HALF:(mh + 1) * HALF],
                        ident4[:],
                        start=True,
                        stop=True,
                    )
                cur = vpool.tile([HALF, 2, NB], F32, tag="hc", name="h0")
                nc.vector.tensor_copy(cur[:], ps_h[:])

                for l in range(L):
                    nxt = vpool.tile([HALF, 2, NB], F32, tag="hc", name=f"h{l + 1}")
                    for mh in range(2):
                        pc = ps_chain.tile(
                            [HALF, NB], F32, tag="ps_c", name=f"ps_c{l}_{mh}"
                        )
                        for kc in range(2):
                            nc.tensor.matmul(
                                pc[:],
                                w_sb[l][:, kc, mh * HALF:(mh + 1) * HALF],
                                cur[:, kc, :],
                                start=(kc == 0),
                                stop=(kc == 1),
                            )
                        bias_ap = bvec[:, 2 * l + mh:2 * l + mh + 1]
                        if l < L - 1:
                            nc.vector.tensor_scalar(
                                nxt[:, mh, :], pc[:], bias_ap, 0.0,
                                mybir.AluOpType.add, mybir.AluOpType.max,
                            )
                        else:
                            nc.vector.tensor_scalar_add(nxt[:, mh, :], pc[:], bias_ap)
                    cur = nxt

                # h3 back to rows [NB, 256], then broadcast to all partitions
                ps_r = ps_row.tile([NB, 2, HALF], F32, tag="ps_r", name="ps_r")
                for mh in range(2):
                    nc.tensor.matmul(
                        ps_r[:, mh, :], cur[:, mh, :], identh[:], start=True, stop=True
                    )
                h3_sb = vpool.tile([NB, 2, HALF], F32, tag="h3", name="h3_sb")
                nc.vector.tensor_copy(h3_sb[:], ps_r[:])
                bc = ps_bc.tile([PP, D], F32, tag="ps_b", name="bc")
                nc.tensor.matmul(
                    bc[:], selbT[:], h3_sb[:].rearrange("b m h -> b (m h)"),
                    start=True, stop=True,
                )

                # residual adds reading bc straight from PSUM (stride-0 t axis)
                for lo, hi, base, npart in (
                    (0, 16, 0, 96), (16, 31, 0, 96),
                    (0, 16, 96, 32), (16, 32, 96, 32),
                ):
                    nk = hi - lo
                    bc_ap = bc[base:base + npart, :].unsqueeze(1).broadcast_to(
                        [npart, nk, D]
                    )
                    sl = slice(base, base + npart)
                    nc.vector.tensor_add(
                        OB[sl, lo * C:hi * C].rearrange("p (t m) -> p t m", m=C),
                        F[sl, lo * C:hi * C].rearrange("p (t m) -> p t m", m=C),
                        bc_ap,
                    )
                engA = STORE_ENGS[0]
                engB = STORE_ENGS[1 % len(STORE_ENGS)]
                getattr(nc, engA).dma_start(outA, OB[0:96, 0:JA])
                getattr(nc, engB).dma_start(outB, OB[96:PP, 0:JB])

            def flat100_body():
                # DMA_ONLY=7: [100, 10240] load+store (phase-free layout probe)
                PP, JJ = 100, NB * N * D // 100
                nf_flat = (
                    nf_d[:].rearrange("b n d -> (b n) d")
                    .rearrange("(p t) d -> p (t d)", p=PP)
                )
                out_flat = (
                    out_d[:].rearrange("b n d -> (b n) d")
                    .rearrange("(p t) d -> p (t d)", p=PP)
                )
                nf_t = dpool.tile([PP, JJ], F32, tag="nfH", name="nfH")
                ob = opool.tile([PP, JJ], out_dt, tag="obH", name="obH")
                nc.vector.memset(ob[:, 0:1], 0.0)
                nc.sync.dma_start(nf_t[:], nf_flat)
                nc.scalar.dma_start(out_flat, ob[:])

            def flat125_body():
                # DMA_ONLY=5: single 125-partition load+store, 32KB contiguous/line
                # DMA_ONLY=6: same but 4x8KB chunks per line (batch-major)
                if DMA_ONLY == 5:
                    nf_flat = (
                        nf_d[:].rearrange("b n d -> (b n) d")
                        .rearrange("(p t) d -> p t d", p=P)
                    )
                    out_flat = (
                        out_d[:].rearrange("b n d -> (b n) d")
                        .rearrange("(p t) d -> p t d", p=P)
                    )
                else:
                    nf_flat = nf_d[:].rearrange("b (p t) d -> p b t d", p=P)
                    out_flat = out_d[:].rearrange("b (p t) d -> p b t d", p=P)
                nf_t = dpool.tile([P, NB * T, D], F32, tag="nfG", name="nfG")
                ob = opool.tile([P, NB * T, D], out_dt, tag="obG", name="obG")
                nc.vector.memset(ob[:, 0, 0:1], 0.0)
                if DMA_ONLY == 6:
                    nc.sync.dma_start(
                        nf_t[:].rearrange("p (b t) d -> p b t d", b=NB), nf_flat
                    )
                    nc.scalar.dma_start(
                        out_flat, ob[:].rearrange("p (b t) d -> p b t d", b=NB)
                    )
                else:
                    nc.sync.dma_start(nf_t[:], nf_flat)
                    nc.scalar.dma_start(out_flat, ob[:])

            def batch_body():
                if DMA_ONLY == 4:
                    flat_body()
                    return
                if DMA_ONLY in (5, 6):
                    flat125_body()
                    return
                for b in range(NB):
                    nf_t = dpool.tile([P, T, D], F32, tag="nf", name=f"nf{b}")
                    src = nf_d[b].rearrange("(p t) d -> p t d", t=T)
                    step = T // LOAD_CHUNKS
                    for s in range(LOAD_CHUNKS):
                        eng = LOAD_ENGS[s % len(LOAD_ENGS)]
                        getattr(nc, eng).dma_start(
                            nf_t[:, s * step:(s + 1) * step, :],
                            src[:, s * step:(s + 1) * step, :],
                        )

                    if DMA_ONLY:
                        # 1: loads+stores, 2: loads only, 3: stores only
                        if DMA_ONLY != 2:
                            ob = opool.tile([P, T, D], out_dt, tag="ob", name=f"ob{b}")
                            nc.vector.memset(ob[:, 0, 0:1], 0.0)
                            dst = out_d[b].rearrange("(p t) d -> p t d", t=T)
                            sstep = T // STORE_CHUNKS
                            for s in range(STORE_CHUNKS):
                                eng = STORE_ENGS[s % len(STORE_ENGS)]
                                getattr(nc, eng).dma_start(
                                    dst[:, s * sstep:(s + 1) * sstep, :],
                                    ob[:, s * sstep:(s + 1) * sstep, :],
                                )
                        continue

                    # per-batch column sums (transposed orientation):
                    # sumT[mh][d, 0] = sum_n nf[b, n, mh*128 + d]
                    h = []
                    for mh in range(2):
                        ps = ps_sum.tile([HALF, 1], F32, tag="ps_s", name=f"ps_s{b}_{mh}")
                        for t in range(T):
                            nc.tensor.matmul(
                                ps[:],
                                nf_t[:, t, mh * HALF:(mh + 1) * HALF],
                                ones_col[:],
                                start=(t == 0),
                                stop=(t == T - 1),
                            )
                        s = vpool.tile([HALF, 1], F32, tag="hT", name=f"sum{b}_{mh}")
                        nc.vector.tensor_scalar_mul(s[:], ps[:], 1.0 / N)
                        h.append(s)

                    # 3-layer chain, transposed orientation, bias+relu on DVE
                    for l in range(L):
                        hn = []
                        for mh in range(2):
                            pc = ps_chain.tile(
                                [HALF, 1], F32, tag="ps_c", name=f"ps_c{b}_{l}_{mh}"
                            )
                            for kc in range(2):
                                nc.tensor.matmul(
                                    pc[:],
                                    w_sb[l][:, kc, mh * HALF:(mh + 1) * HALF],
                                    h[kc][:],
                                    start=(kc == 0),
                                    stop=(kc == 1),
                                )
                            ht = vpool.tile([HALF, 1], F32, tag="hT", name=f"h{b}_{l}_{mh}")
                            bias_ap = bvec[:, 2 * l + mh:2 * l + mh + 1]
                            if l < L - 1:
                                nc.vector.tensor_scalar(
                                    ht[:], pc[:], bias_ap, 0.0, add_op, max_op
                                )
                            else:
                                nc.vector.tensor_scalar_add(ht[:], pc[:], bias_ap)
                            hn.append(ht)
                        h = hn

                    # transpose h3 back to a row, broadcast across partitions
                    pr = ps_row.tile([1, D], F32, tag="ps_r", name=f"ps_r{b}")
                    for kc in range(2):
                        nc.tensor.transpose(
                            pr[0:1, kc * HALF:(kc + 1) * HALF], h[kc][:], ident[:]
                        )
                    h3row = vpool.tile([1, D], F32, tag="h3row", name=f"h3row{b}")
                    nc.vector.tensor_copy(h3row[:], pr[:])
                    pb = ps_bc.tile([P, D], F32, tag="ps_b", name=f"ps_b{b}")
                    nc.tensor.matmul(pb[:], ones_row[:], h3row[:], start=True, stop=True)

                    # residual add (fp32 + fp32 -> out_dt) reading the broadcast
                    # straight from PSUM with a stride-0 AP over the t axis;
                    # store each chunk as soon as its adds complete
                    ob = opool.tile([P, T, D], out_dt, tag="ob", name=f"ob{b}")
                    dst = out_d[b].rearrange("(p t) d -> p t d", t=T)
                    astep = T // ADD_CHUNKS
                    sstep = T // STORE_CHUNKS
                    # gpsimd can't read PSUM: give it an SBUF copy of the bc row
                    bc_sb = None
                    if any(e != "vector" for e in ADD_ENGS):
                        bc_sb = vpool.tile([P, D], F32, tag="bc_sb", name=f"bc_sb{b}")
                        nc.vector.tensor_copy(bc_sb[:], pb[:])
                    adds_done = 0
                    for s in range(STORE_CHUNKS):
                        hi = (s + 1) * sstep
                        while adds_done < hi:
                            a0 = adds_done
                            eng = ADD_ENGS[(adds_done // astep) % len(ADD_ENGS)]
                            src_bc = pb if eng == "vector" else bc_sb
                            pb_bc = src_bc[:].unsqueeze(1).broadcast_to([P, astep, D])
                            getattr(nc, eng).tensor_add(
                                ob[:, a0:a0 + astep, :], nf_t[:, a0:a0 + astep, :], pb_bc
                            )
                            adds_done += astep
                        eng = STORE_ENGS[s % len(STORE_ENGS)]
                        getattr(nc, eng).dma_start(
                            dst[:, s * sstep:hi, :], ob[:, s * sstep:hi, :]
                        )

            if V3 == 3:
                body = lambda: v5_body(consts)
            elif V3 == 2:
                body = lambda: v4_body(consts)
            elif V3:
                body = lambda: v3_body(consts)
            else:
                body = batch_body
            u = UNROLL if reps % UNROLL == 0 else 1
            loops = reps // u
            if loops == 1:
                for _ in range(u):
                    body()
            else:
                with tc.For_i(0, loops, 1):
                    for _ in range(u):
                        body()

    nc.compile()
    return nc


def _get_nc(reps=1):
    if reps not in _NC_CACHE:
        _NC_CACHE[reps] = _build_nc(reps)
    return _NC_CACHE[reps]


def _make_in_maps(node_feature, Ws, bs):
    nf = np.ascontiguousarray(np.asarray(node_feature, dtype=np.float32))
    w = np.ascontiguousarray(np.asarray(Ws, dtype=np.float32))
    if V3:
        w = w.copy()
        w[0] *= 1.0 / N  # fold the node-mean 1/N into W0
    b = np.asarray(bs, dtype=np.float32)
    # bvec[p, 2*l + half] = bs[l, half*128 + p]
    bvec = np.ascontiguousarray(
        b.reshape(L, 2, HALF).transpose(2, 0, 1).reshape(HALF, 2 * L)
    )
    in_maps = []
    for i in range(NCORES):
        in_maps.append(
            {
                "nf": np.ascontiguousarray(nf[i * NB:(i + 1) * NB]),
                "w": w,
                "bvec": bvec,
            }
        )
    return in_maps


def run_on_hw(node_feature, Ws, bs):
    # The NTFF trace hook (antenv.axon_hooks) does not exist in this
    # container; make sure an inherited BASS_TRACE can't pull it in.
    os.environ["BASS_NEVER_TRACE"] = "1"
    nc = _get_nc()
    res = run_bass_kernel_spmd(
        nc,
        _make_in_maps(node_feature, Ws, bs),
        list(range(NCORES)),
        trace=False,
    )
    out = np.concatenate(
        [np.asarray(res.results[i]["out"], dtype=np.float32) for i in range(NCORES)],
        axis=0,
    )
    return out, res


def kernel(x, node_feature, Ws, bs):
    node_feature = np.asarray(node_feature, dtype=np.float32)
    out, _ = run_on_hw(node_feature, Ws, bs)
    return out, node_feature


# ---------------------------------------------------------------------------
# Timing runner: same PJRT path as run_bass_kernel_spmd under axon, but with
# the jitted executable cached so repeated executions can be timed without
# re-tracing/re-compiling. Used by test.py only.
# ---------------------------------------------------------------------------


class _Runner:
    def __init__(self, nc=None):
        import jax
        from jax.experimental.shard_map import shard_map
        from jax.sharding import Mesh, NamedSharding, PartitionSpec

        from concourse.bass2jax import (
            _bass_exec_p,
            install_neuronx_cc_hook,
            partition_id_tensor,
        )

        install_neuronx_cc_hook()
        self.jax = jax
        if nc is None:
            nc = _get_nc(1)
        partition_name = (
            nc.partition_id_tensor.name if nc.partition_id_tensor else None
        )
        in_names, out_names, out_avals, zero_outs = [], [], [], []
        for alloc in nc.m.functions[0].allocations:
            if not isinstance(alloc, mybir.MemoryLocationSet):
                continue
            name = alloc.memorylocations[0].name
            if alloc.kind == "ExternalInput":
                if name != partition_name:
                    in_names.append(name)
            elif alloc.kind == "ExternalOutput":
                shape = tuple(alloc.tensor_shape)
                dt = mybir.dt.np(alloc.dtype)
                out_names.append(name)
                out_avals.append(jax.core.ShapedArray(shape, dt))
                zero_outs.append(np.zeros(shape, dt))
        self.in_names = in_names
        self.out_names = out_names
        self.out_avals = out_avals
        self.zero_outs = zero_outs
        n_params, n_outs = len(in_names), len(out_names)
        all_names = tuple(
            in_names + out_names + ([partition_name] if partition_name else [])
        )

        def _body(*args):
            operands = list(args)
            if partition_name is not None:
                operands.append(partition_id_tensor())
            outs = _bass_exec_p.bind(
                *operands,
                out_avals=tuple(out_avals),
                in_names=all_names,
                out_names=tuple(out_names),
                lowering_input_output_aliases=(),
                sim_require_finite=True,
                sim_require_nnan=True,
                nc=nc,
            )
            return tuple(outs)

        devices = jax.devices()[:NCORES]
        self.mesh = Mesh(np.asarray(devices), ("core",))
        self.sharding = NamedSharding(self.mesh, PartitionSpec("core"))
        in_specs = (PartitionSpec("core"),) * (n_params + n_outs)
        out_specs = (PartitionSpec("core"),) * n_outs
        self.jitted = jax.jit(
            shard_map(
                _body,
                mesh=self.mesh,
                in_specs=in_specs,
                out_specs=out_specs,
                check_rep=False,
            ),
            donate_argnums=tuple(range(n_params, n_params + n_outs)),
            keep_unused=True,
        )

    def stage_inputs(self, in_maps):
        concat = [
            np.concatenate([m[name] for m in in_maps], axis=0)
            for name in self.in_names
        ]
        return [self.jax.device_put(a, self.sharding) for a in concat]

    def stage_zeros(self):
        return [
            self.jax.device_put(
                np.zeros((NCORES * z.shape[0], *z.shape[1:]), z.dtype), self.sharding
            )
            for z in self.zero_outs
        ]

    def run(self, dev_inputs, dev_zeros):
        return self.jitted(*dev_inputs, *dev_zeros)


_RUNNER_CACHE = {}


def get_runner(reps=1):
    if reps not in _RUNNER_CACHE:
        _RUNNER_CACHE[reps] = _Runner(_get_nc(reps))
    return _RUNNER_CACHE[reps]
